# revision 1
# baseline (speedup 1.0000x reference)
# Trainium2 Bass kernel for ChannelAttentionBlock (B=8,C=256,H=W=128,S=64,HEADS=8)
# Data-parallel over batch: 1 sample per NeuronCore, 8 cores.
import numpy as np
import ml_dtypes

import concourse.bass as bass
from concourse import bacc
import concourse.mybir as mybir
from concourse.bass_utils import run_bass_kernel_spmd
from concourse.tile import TileContext

F32R = mybir.dt.float32r
F32 = mybir.dt.float32
BF16 = mybir.dt.bfloat16
AF = mybir.ActivationFunctionType
ALU = mybir.AluOpType

B, C, H, W = 8, 256, 128, 128
S = 64
HEADS = 8
HW = H * W
WP = W + 2          # padded row width
R = 16              # rows per strip
NSTRIP = H // R
BLK_ROWS = 4
NBLK = 4
NPX = BLK_ROWS * W  # 512
EPS = 1e-5

TAPS = [(dy, dx) for dy in (-1, 0, 1) for dx in (-1, 0, 1)]

_CACHED = {}


def build_nc(debug_taps=False):
    nc = bacc.Bacc("TRN2", target_bir_lowering=False, debug=False)

    # ------------- DRAM parameters (host layouts) -------------
    # x/y: [128(part), 2(ktile), H+4(rows: img rows -2..129), WP]
    x_in = nc.dram_tensor("x", [128, 2, H + 4, WP], F32R, kind="ExternalInput")
    y_in = nc.dram_tensor("y", [128, 2, H + 4, WP], F32R, kind="ExternalInput")
    wcq_in = nc.dram_tensor("wcq", [128, 2, 9, S + 1], F32R, kind="ExternalInput")
    wckv_in = nc.dram_tensor("wckv", [128, 2, 9, S + 1], F32R, kind="ExternalInput")
    wqkv_in = nc.dram_tensor("wqkv", [128, 9, 128], F32R, kind="ExternalInput")
    wpo_in = nc.dram_tensor("wpo", [S, S], F32R, kind="ExternalInput")
    wexp_in = nc.dram_tensor("wexp", [S, 9, C], BF16, kind="ExternalInput")
    wf1x_in = nc.dram_tensor("wf1x", [128, 2, 2, 128], F32R, kind="ExternalInput")
    wf1v_in = nc.dram_tensor("wf1v", [128, 2, 2, 128], F32R, kind="ExternalInput")
    wdw_in = nc.dram_tensor("wdw", [128, 2, 9, 128], F32R, kind="ExternalInput")
    wf2_in = nc.dram_tensor("wf2", [128, 2, 2, 128], F32R, kind="ExternalInput")
    stat_cq_in = nc.dram_tensor("stat_cq", [S + 1, 1], F32R, kind="ExternalInput")
    bc2a_in = nc.dram_tensor("bc2a", [1, 128], F32R, kind="ExternalInput")    # 1s at rows0-63
    bc2b_in = nc.dram_tensor("bc2b", [1, 128], F32R, kind="ExternalInput")    # 1s at rows64-127
    ones128_in = nc.dram_tensor("ones128", [1, 128], F32R, kind="ExternalInput")
    stat256_in = nc.dram_tensor("stat256", [128, 2], F32R, kind="ExternalInput")
    bias_q_in = nc.dram_tensor("bias_q", [S, 1], F32, kind="ExternalInput")
    bias_kv_in = nc.dram_tensor("bias_kv", [2 * S, 1], F32, kind="ExternalInput")
    bias_g_in = nc.dram_tensor("bias_g", [128, 2, 1], F32, kind="ExternalInput")
    temp_in = nc.dram_tensor("tempv", [S, 1], F32, kind="ExternalInput")
    mask_in = nc.dram_tensor("maskbd", [S, S], F32R, kind="ExternalInput")
    ident_in = nc.dram_tensor("ident", [128, 128], BF16, kind="ExternalInput")
    identr_in = nc.dram_tensor("identr", [S, S], F32R, kind="ExternalInput")
    ones_in = nc.dram_tensor("onesr", [1, S], F32R, kind="ExternalInput")
    zeros_in = nc.dram_tensor("zeros", [128, 2600], F32R, kind="ExternalInput")
    zerosb_in = nc.dram_tensor("zerosb", [128, 2600], BF16, kind="ExternalInput")

    fx_dram = nc.dram_tensor("fx_dram", [2, 128, HW], BF16)
    out_dram = nc.dram_tensor("out", [2, 128, HW], F32, kind="ExternalOutput")
    if debug_taps:
        dbg_xq = nc.dram_tensor("dbg_xq", [128, HW], F32, kind="ExternalOutput")
        dbg_qk = nc.dram_tensor("dbg_qk", [128, HW], F32, kind="ExternalOutput")
        dbg_vo = nc.dram_tensor("dbg_vo", [128, HW], F32, kind="ExternalOutput")
        dbg_at = nc.dram_tensor("dbg_at", [S, S + 2], F32, kind="ExternalOutput")
        dbg_f1 = nc.dram_tensor("dbg_f1", [128, 2, HW], F32, kind="ExternalOutput")

    with TileContext(nc) as tc:
        with tc.tile_pool(name="persist", bufs=1) as persist:
            qk_store = persist.tile([128, HW], BF16, tag="qk_store")
            vo_store = persist.tile([128, HW], BF16, tag="vo_store")
            rq2 = persist.tile([S, 1], F32, tag="rq2")
            rk2 = persist.tile([S, 1], F32, tag="rk2")
            nc.vector.memset(rq2, 0.0)
            nc.vector.memset(rk2, 0.0)

            # ================= PHASE 1 =================
            with tc.tile_pool(name="p1w", bufs=1) as p1w, \
                 tc.tile_pool(name="p1", bufs=2) as p1, \
                 tc.tile_pool(name="p1ps", bufs=1, space="PSUM") as p1ps:
                wcq = p1w.tile([128, 2, 9, S + 1], F32R, tag="wcq")
                nc.sync.dma_start(out=wcq, in_=wcq_in[:, :, :, :])
                wckv = p1w.tile([128, 2, 9, S + 1], F32R, tag="wckv")
                nc.sync.dma_start(out=wckv, in_=wckv_in[:, :, :, :])
                wqkv = p1w.tile([128, 9, 128], F32R, tag="wqkv")
                nc.sync.dma_start(out=wqkv, in_=wqkv_in[:, :, :])
                wf1x = p1w.tile([128, 2, 2, 128], F32R, tag="wf1x")
                nc.sync.dma_start(out=wf1x, in_=wf1x_in[:, :, :, :])
                stat_cq = p1w.tile([S + 1, 1], F32R, tag="stat_cq")
                nc.sync.dma_start(out=stat_cq, in_=stat_cq_in[:, :])
                bc2a = p1w.tile([1, 128], F32R, tag="bc2a")
                nc.sync.dma_start(out=bc2a, in_=bc2a_in[:, :])
                bc2b = p1w.tile([1, 128], F32R, tag="bc2b")
                nc.sync.dma_start(out=bc2b, in_=bc2b_in[:, :])
                bias_q = p1w.tile([S, 1], F32, tag="bias_q")
                nc.sync.dma_start(out=bias_q, in_=bias_q_in[:, :])
                bias_kv = p1w.tile([2 * S, 1], F32, tag="bias_kv")
                nc.sync.dma_start(out=bias_kv, in_=bias_kv_in[:, :])

                xwin = p1w.tile([128, 2, 18, WP], F32R, tag="xwin")
                ywin = p1w.tile([128, 2, 18, WP], F32R, tag="ywin")
                # nwin: xq rows0-63 / ykv rows64-127 ; slot i = row (r0-2)+i ; slot18 always zero
                nwin = p1w.tile([128, 19, WP], F32R, tag="nwin")
                nc.sync.dma_start(out=nwin.rearrange("p a b -> p (a b)"),
                                  in_=zeros_in[:, :19 * WP])

                def q_kv_convs(rq, nrows, sl_base):
                    """q & kv convs for q-rows rq..rq+nrows-1 ; nwin slot of row rq is sl_base."""
                    npx_q = nrows * W
                    ps_q = p1ps.tile([128, NPX], F32, tag="ps_q")
                    ps_kv = p1ps.tile([128, NPX], F32, tag="ps_kv")
                    for t_i, (dy, dx) in enumerate(TAPS):
                        sl0 = sl_base + dy
                        rhs_q = nwin[0:64, sl0:sl0 + nrows, 1 + dx:1 + dx + W]
                        nc.tensor.matmul(ps_q[0:S, 0:npx_q], wqkv[0:64, t_i, 0:S], rhs_q,
                                         start=(t_i == 0), stop=(t_i == 8))
                        rhs_kv = nwin[64:128, sl0:sl0 + nrows, 1 + dx:1 + dx + W]
                        nc.tensor.matmul(ps_kv[:, 0:npx_q], wqkv[64:128, t_i], rhs_kv,
                                         start=(t_i == 0), stop=(t_i == 8))
                    q_acc = p1.tile([S, 1], F32, tag="q_acc")
                    k_acc = p1.tile([S, 1], F32, tag="k_acc")
                    qsb = p1.tile([S, NPX], F32, tag="qsb")
                    qsq = p1.tile([S, NPX], F32, tag="qsq")
                    nc.scalar.activation(qsb[:, 0:npx_q], ps_q[0:S, 0:npx_q], AF.Identity, bias=bias_q)
                    nc.vector.tensor_copy(qk_store[0:S, rq * W:rq * W + npx_q], qsb[:, 0:npx_q])
                    nc.scalar.activation(qsq[:, 0:npx_q], qsb[:, 0:npx_q], AF.Square, accum_out=q_acc)
                    nc.vector.tensor_tensor(rq2, rq2, q_acc, op=ALU.add)
                    kvsb = p1.tile([128, NPX], F32, tag="kvsb")
                    ksq = p1.tile([S, NPX], F32, tag="ksq")
                    nc.scalar.activation(kvsb[:, 0:npx_q], ps_kv[:, 0:npx_q], AF.Identity, bias=bias_kv)
                    nc.vector.tensor_copy(qk_store[64:128, rq * W:rq * W + npx_q], kvsb[0:S, 0:npx_q])
                    nc.vector.tensor_copy(vo_store[0:S, rq * W:rq * W + npx_q], kvsb[S:2 * S, 0:npx_q])
                    nc.scalar.activation(ksq[:, 0:npx_q], kvsb[0:S, 0:npx_q], AF.Square, accum_out=k_acc)
                    nc.vector.tensor_tensor(rk2, rk2, k_acc, op=ALU.add)

                for s_i in range(NSTRIP):
                    r0 = 16 * s_i
                    if s_i > 0:
                        nc.vector.tensor_copy(xwin[:, :, 0:2], xwin[:, :, 16:18])
                        nc.vector.tensor_copy(ywin[:, :, 0:2], ywin[:, :, 16:18])
                        nc.vector.tensor_copy(nwin[:, 0:2], nwin[:, 16:18])
                    for b_i in range(NBLK):
                        rb = r0 + BLK_ROWS * b_i
                        # xwin slot i = row (r0-1)+i  (18 slots: rows r0-1..r0+16)
                        if s_i == 0 and b_i == 0:
                            nc.sync.dma_start(out=xwin[:, :, 0:6], in_=x_in[:, :, 1:7])
                            nc.sync.dma_start(out=ywin[:, :, 0:6], in_=y_in[:, :, 1:7])
                        else:
                            sl = 4 * b_i + 2
                            nc.sync.dma_start(out=xwin[:, :, sl:sl + 4], in_=x_in[:, :, rb + 3:rb + 7])
                            nc.sync.dma_start(out=ywin[:, :, sl:sl + 4], in_=y_in[:, :, rb + 3:rb + 7])
                        # ---- cq / ckv conv + LN -> nwin rows (xq: 0-63, ykv: 64-127)
                        for (name, wl, win_t, dst_lo) in (("cq", wcq, xwin, 0), ("ckv", wckv, ywin, 64)):
                            ps_c = p1ps.tile([128, NPX], F32, tag=f"ps_{name}")
                            first = True
                            for kt in range(2):
                                for t_i, (dy, dx) in enumerate(TAPS):
                                    sl0 = 4 * b_i + 1 + dy
                                    rhs = win_t[:, kt, sl0:sl0 + 4, 1 + dx:1 + dx + W]
                                    nc.tensor.matmul(ps_c[0:S + 1], wl[:, kt, t_i], rhs,
                                                     start=first, stop=(kt == 1 and t_i == 8))
                                    first = False
                            t_sb = p1.tile([S, NPX], F32, tag=f"t_{name}")
                            nc.scalar.copy(t_sb, ps_c[0:S])
                            sq_sb = p1.tile([S + 1, NPX], F32R, tag=f"sq_{name}")
                            nc.scalar.activation(sq_sb, ps_c[0:S + 1], AF.Square)
                            ps_v = p1ps.tile([1, NPX], F32, tag="ps_v")
                            nc.tensor.matmul(ps_v[0:1], stat_cq, sq_sb, start=True, stop=True)
                            mu_t = p1.tile([1, NPX], F32R, tag=f"mut_{name}", bufs=1)
                            nc.scalar.mul(mu_t, ps_c[64:65], 1.0 / S)
                            varr = p1.tile([1, NPX], F32, tag=f"var_{name}", bufs=1)
                            nc.vector.tensor_scalar_add(varr, ps_v[0:1], EPS)
                            rcpv = p1.tile([1, NPX], F32, tag=f"rcp_{name}", bufs=1)
                            nc.vector.reciprocal_approx_fast(out=rcpv, in_=varr)
                            r_t = p1.tile([1, NPX], F32R, tag=f"rt_{name}", bufs=1)
                            nc.scalar.activation(r_t, rcpv, AF.Sqrt)
                            ps_b = p1ps.tile([128, NPX], F32, tag="ps_b")
                            nc.tensor.matmul(ps_b, bc2a, mu_t, start=True, stop=False)
                            nc.tensor.matmul(ps_b, bc2b, r_t, start=False, stop=True)
                            d_sb = p1.tile([S, NPX], F32, tag=f"d_{name}")
                            nc.vector.tensor_tensor(d_sb, t_sb, ps_b[0:S], op=ALU.subtract)
                            dst = nwin[dst_lo:dst_lo + 64, 4 * b_i + 2:4 * b_i + 6, 1:1 + W]
                            nc.vector.tensor_tensor(dst,
                                                    d_sb.rearrange("p (a b) -> p a b", a=4),
                                                    ps_b[64:128].rearrange("p (a b) -> p a b", a=4),
                                                    op=ALU.mult)
                        # ---- fx (ffn1 x-half) rows rb..rb+3
                        for mt in range(2):
                            ps_fx = p1ps.tile([128, NPX], F32, tag="ps_fx")
                            for kt in range(2):
                                rhs = xwin[:, kt, 4 * b_i + 1:4 * b_i + 5, 1:1 + W]
                                nc.tensor.matmul(ps_fx, wf1x[:, kt, mt], rhs,
                                                 start=(kt == 0), stop=(kt == 1))
                            fx_sb = p1.tile([128, NPX], BF16, tag="fx_sb")
                            nc.scalar.copy(fx_sb, ps_fx)
                            nc.sync.dma_start(out=fx_dram[mt, :, rb * W:(rb + 4) * W], in_=fx_sb)
                        # ---- q / kv convs (lag 1 row)
                        if s_i == 0 and b_i == 0:
                            q_kv_convs(0, 3, 2)
                        else:
                            q_kv_convs(rb - 1, 4, 4 * b_i + 1)
                # epilogue: q/kv row 127 (nwin slot of row r = r-110 ; slot18 zero)
                q_kv_convs(127, 1, 17)
                if debug_taps:
                    xq_f = p1.tile([128, HW], F32, tag="xq_f", bufs=1)
                    nc.vector.tensor_copy(xq_f[:, 0:HW], nwin[:, 2:18, 1:1 + W].rearrange("p a b -> p (a b)"))
                    nc.sync.dma_start(out=dbg_xq[:, 14 * 16 * W:HW], in_=xq_f[:, 0:2 * 16 * W])

            # ================= PHASE 2: attention =================
            with tc.tile_pool(name="p2", bufs=2) as p2, \
                 tc.tile_pool(name="p2one", bufs=1) as p2one, \
                 tc.tile_pool(name="p2ps", bufs=2, space="PSUM") as p2ps:
                ident = p2one.tile([128, 128], BF16, tag="ident")
                nc.sync.dma_start(out=ident, in_=ident_in[:, :])
                g_ps = p2ps.tile([S, S], F32, tag="g_ps", bufs=1)
                for tb in range(HW // 128):
                    tp = p2ps.tile([128, 128], BF16, tag="tp")
                    nc.tensor.transpose(tp, qk_store[:, tb * 128:(tb + 1) * 128], ident)
                    tp_sb = p2.tile([128, 128], BF16, tag="tp_sb")
                    nc.scalar.copy(tp_sb, tp)
                    nc.tensor.matmul(g_ps, tp_sb[:, 0:S], tp_sb[:, 64:128],
                                     start=(tb == 0), stop=(tb == HW // 128 - 1))
                g_sb = p2one.tile([S, S], F32, tag="g_sb")
                nc.scalar.copy(g_sb, g_ps)
                rqs = p2one.tile([S, 1], F32, tag="rqs")
                rks = p2one.tile([S, 1], F32, tag="rks")
                sq1 = p2one.tile([S, 1], F32, tag="sq1")
                sq2 = p2one.tile([S, 1], F32, tag="sq2")
                nc.vector.reciprocal_approx_fast(out=sq1, in_=rq2)
                nc.scalar.activation(rqs, sq1, AF.Sqrt)
                nc.vector.reciprocal_approx_fast(out=sq2, in_=rk2)
                nc.scalar.activation(rks, sq2, AF.Sqrt)
                temp_t = p2one.tile([S, 1], F32, tag="temp_t")
                nc.sync.dma_start(out=temp_t, in_=temp_in[:, :])
                nc.vector.tensor_tensor(rqs, rqs, temp_t, op=ALU.mult)
                nc.vector.tensor_scalar_mul(g_sb, g_sb, rqs)
                rk_row = p2one.tile([1, S], F32R, tag="rk_row")
                nc.sync.dma_start(out=rk_row, in_=rks[:, :].bitcast(F32R))
                ones1 = p2one.tile([1, S], F32R, tag="ones1")
                nc.sync.dma_start(out=ones1, in_=ones_in[:, :])
                rkb_ps = p2ps.tile([S, S], F32, tag="rkb_ps", bufs=1)
                nc.tensor.matmul(rkb_ps, ones1, rk_row, start=True, stop=True)
                s_sb = p2one.tile([S, 8, 8], F32, tag="s_sb")
                nc.vector.tensor_tensor(s_sb.rearrange("p a b -> p (a b)"), g_sb, rkb_ps, op=ALU.mult)
                mx = p2one.tile([S, 8], F32, tag="mx")
                nc.vector.reduce_max(mx, s_sb, axis=mybir.AxisListType.X)
                mxb = bass.AP(tensor=mx.tensor, offset=mx.offset,
                              ap=[list(mx.ap[0]), list(mx.ap[1]), [0, 8]])
                e_sb = p2one.tile([S, 8, 8], F32, tag="e_sb")
                nc.vector.tensor_tensor(e_sb, s_sb, mxb, op=ALU.subtract)
                ex_sb = p2one.tile([S, 8, 8], F32, tag="ex_sb")
                nc.scalar.activation(ex_sb, e_sb, AF.Exp)
                sm = p2one.tile([S, 8], F32, tag="sm")
                nc.vector.reduce_sum(sm, ex_sb, axis=mybir.AxisListType.X)
                rs = p2one.tile([S, 8], F32, tag="rs")
                nc.vector.reciprocal_approx_fast(out=rs, in_=sm)
                rsb = bass.AP(tensor=rs.tensor, offset=rs.offset,
                              ap=[list(rs.ap[0]), list(rs.ap[1]), [0, 8]])
                attn = p2one.tile([S, S], F32R, tag="attn")
                nc.vector.tensor_tensor(attn.rearrange("p (a b) -> p a b", a=8), ex_sb, rsb, op=ALU.mult)
                maskbd = p2one.tile([S, S], F32R, tag="maskbd")
                nc.sync.dma_start(out=maskbd, in_=mask_in[:, :])
                attn_m = p2one.tile([S, S], F32R, tag="attn_m")
                nc.vector.tensor_tensor(attn_m, attn, maskbd, op=ALU.mult)
                identr = p2one.tile([S, S], F32R, tag="identr")
                nc.sync.dma_start(out=identr, in_=identr_in[:, :])
                attn_tp = p2ps.tile([S, S], F32R, tag="attn_tp", bufs=1)
                nc.tensor.transpose(attn_tp, attn_m, identr)
                attn_t = p2one.tile([S, S], F32R, tag="attn_t")
                nc.scalar.copy(attn_t, attn_tp)
                if debug_taps:
                    at_f = p2one.tile([S, S + 2], F32, tag="at_f")
                    nc.vector.tensor_copy(at_f[:, 0:S], attn_m[:, :].bitcast(F32))
                    nc.vector.tensor_copy(at_f[:, S:S + 1], rqs)
                    nc.vector.tensor_copy(at_f[:, S + 1:S + 2], rks)
                    nc.sync.dma_start(out=dbg_at[:, :], in_=at_f)
                wpo = p2one.tile([S, S], F32R, tag="wpo")
                nc.sync.dma_start(out=wpo, in_=wpo_in[:, :])
                for blk in range(HW // NPX):
                    vsb = p2.tile([S, NPX], F32R, tag="vsb")
                    nc.vector.tensor_copy(vsb, vo_store[0:S, blk * NPX:(blk + 1) * NPX])
                    ps_o = p2ps.tile([S, NPX], F32, tag="ps_o", bufs=1)
                    nc.tensor.matmul(ps_o, attn_t, vsb, start=True, stop=True)
                    o_sb = p2.tile([S, NPX], F32R, tag="o_sb")
                    nc.scalar.copy(o_sb, ps_o)
                    ps_po = p2ps.tile([S, NPX], F32, tag="ps_po", bufs=1)
                    nc.tensor.matmul(ps_po, wpo, o_sb, start=True, stop=True)
                    nc.vector.tensor_copy(vo_store[64:128, blk * NPX:(blk + 1) * NPX], ps_po)
                if debug_taps:
                    for half in range(2):
                        qk_f = p2.tile([128, HW // 2], F32, tag="qk_f", bufs=1)
                        nc.vector.tensor_copy(qk_f, qk_store[:, half * HW // 2:(half + 1) * HW // 2])
                        nc.sync.dma_start(out=dbg_qk[:, half * HW // 2:(half + 1) * HW // 2], in_=qk_f)
                        vo_f = p2.tile([128, HW // 2], F32, tag="vo_f", bufs=1)
                        nc.vector.tensor_copy(vo_f, vo_store[:, half * HW // 2:(half + 1) * HW // 2])
                        nc.sync.dma_start(out=dbg_vo[:, half * HW // 2:(half + 1) * HW // 2], in_=vo_f)

            # ================= PHASE 3: expand + LN + FFN =================
            with tc.tile_pool(name="p3w", bufs=1) as p3w, \
                 tc.tile_pool(name="p3", bufs=2) as p3, \
                 tc.tile_pool(name="p3ps", bufs=1, space="PSUM") as p3ps:
                wexp = p3w.tile([S, 9, C], BF16, tag="wexp")
                nc.sync.dma_start(out=wexp, in_=wexp_in[:, :, :])
                wf1v = p3w.tile([128, 2, 2, 128], F32R, tag="wf1v")
                nc.sync.dma_start(out=wf1v, in_=wf1v_in[:, :, :, :])
                wdw = p3w.tile([128, 2, 9, 128], F32R, tag="wdw")
                nc.sync.dma_start(out=wdw, in_=wdw_in[:, :, :, :])
                wf2 = p3w.tile([128, 2, 2, 128], F32R, tag="wf2")
                nc.sync.dma_start(out=wf2, in_=wf2_in[:, :, :, :])
                stat256 = p3w.tile([128, 2], F32R, tag="stat256")
                nc.sync.dma_start(out=stat256, in_=stat256_in[:, :])
                ones128 = p3w.tile([1, 128], F32R, tag="ones128")
                nc.sync.dma_start(out=ones128, in_=ones128_in[:, :])
                bias_g = p3w.tile([128, 2, 1], F32, tag="bias_g")
                nc.sync.dma_start(out=bias_g, in_=bias_g_in[:, :, :])
                # owin: slot i = o row (r0-2)+i ; slot18 zero
                owin = p3w.tile([S, 19, WP], BF16, tag="owin")
                nc.sync.dma_start(out=owin.rearrange("p a b -> p (a b)"), in_=zerosb_in[0:S, :19 * WP])
                # f1win: slot i = f1 row (r0-3)+i (slots 0..18); slot19 always zero
                f1win = p3w.tile([128, 2, 20, WP], F32R, tag="f1win")
                for half in range(2):
                    nc.sync.dma_start(out=f1win[:, half].rearrange("p a b -> p (a b)"),
                                      in_=zeros_in[:, :20 * WP])

                def stage_a(re, nrows, slo, b_i, s_i):
                    """expand conv rows re..re+nrows-1 (owin slot of row re = slo) + LN + ffn1 -> f1win"""
                    npx_e = nrows * W
                    ps_e0 = p3ps.tile([128, NPX], F32, tag="ps_e0")
                    ps_e1 = p3ps.tile([128, NPX], F32, tag="ps_e1")
                    for t_i, (dy, dx) in enumerate(TAPS):
                        sl0 = slo + dy
                        rhs = owin[:, sl0:sl0 + nrows, 1 + dx:1 + dx + W]
                        nc.tensor.matmul(ps_e0[:, 0:npx_e], wexp[:, t_i, 0:128], rhs,
                                         start=(t_i == 0), stop=(t_i == 8))
                        nc.tensor.matmul(ps_e1[:, 0:npx_e], wexp[:, t_i, 128:256], rhs,
                                         start=(t_i == 0), stop=(t_i == 8))
                    t0 = p3.tile([128, NPX], F32R, tag="t0")
                    t1 = p3.tile([128, NPX], F32R, tag="t1")
                    nc.scalar.copy(t0[:, 0:npx_e], ps_e0[:, 0:npx_e])
                    nc.scalar.copy(t1[:, 0:npx_e], ps_e1[:, 0:npx_e])
                    sq0 = p3.tile([128, NPX], F32R, tag="sq0")
                    sq1t = p3.tile([128, NPX], F32R, tag="sq1t")
                    nc.scalar.activation(sq0[:, 0:npx_e], ps_e0[:, 0:npx_e], AF.Square)
                    nc.scalar.activation(sq1t[:, 0:npx_e], ps_e1[:, 0:npx_e], AF.Square)
                    ps_stm = p3ps.tile([1, NPX], F32, tag="small", bufs=2)
                    nc.tensor.matmul(ps_stm[0:1, 0:npx_e], stat256[:, 0:1], t0[:, 0:npx_e], start=True, stop=False)
                    nc.tensor.matmul(ps_stm[0:1, 0:npx_e], stat256[:, 0:1], t1[:, 0:npx_e], start=False, stop=True)
                    ps_sts = p3ps.tile([1, NPX], F32, tag="small", bufs=2)
                    nc.tensor.matmul(ps_sts[0:1, 0:npx_e], stat256[:, 1:2], sq0[:, 0:npx_e], start=True, stop=False)
                    nc.tensor.matmul(ps_sts[0:1, 0:npx_e], stat256[:, 1:2], sq1t[:, 0:npx_e], start=False, stop=True)
                    mu3 = p3.tile([1, NPX], F32R, tag="mu3", bufs=1)
                    nc.scalar.copy(mu3[:, 0:npx_e], ps_stm[0:1, 0:npx_e])
                    musq = p3.tile([1, NPX], F32, tag="musq", bufs=1)
                    mu3v = mu3[:, 0:npx_e].bitcast(F32)
                    nc.vector.tensor_tensor(musq[:, 0:npx_e], mu3v, mu3v, op=ALU.mult)
                    varr = p3.tile([1, NPX], F32, tag="varr", bufs=1)
                    nc.vector.scalar_tensor_tensor(varr[:, 0:npx_e], ps_sts[0:1, 0:npx_e], EPS,
                                                   musq[:, 0:npx_e], op0=ALU.add, op1=ALU.subtract)
                    rcpv = p3.tile([1, NPX], F32, tag="rcpv", bufs=1)
                    nc.vector.reciprocal_approx_fast(out=rcpv[:, 0:npx_e], in_=varr[:, 0:npx_e])
                    r3 = p3.tile([1, NPX], F32R, tag="r3", bufs=1)
                    nc.scalar.activation(r3[:, 0:npx_e], rcpv[:, 0:npx_e], AF.Sqrt)
                    ps_mu = p3ps.tile([128, NPX], F32, tag="small", bufs=2)
                    nc.tensor.matmul(ps_mu[:, 0:npx_e], ones128, mu3[:, 0:npx_e], start=True, stop=True)
                    ps_r = p3ps.tile([128, NPX], F32, tag="small", bufs=2)
                    nc.tensor.matmul(ps_r[:, 0:npx_e], ones128, r3[:, 0:npx_e], start=True, stop=True)
                    vn0 = p3.tile([128, NPX], F32R, tag="vn0")
                    vn1 = p3.tile([128, NPX], F32R, tag="vn1")
                    for vt, tt in ((vn0, t0), (vn1, t1)):
                        dsb = p3.tile([128, NPX], F32, tag="dsb")
                        nc.vector.tensor_tensor(dsb[:, 0:npx_e], tt[:, 0:npx_e], ps_mu[:, 0:npx_e], op=ALU.subtract)
                        nc.vector.tensor_tensor(vt[:, 0:npx_e], dsb[:, 0:npx_e], ps_r[:, 0:npx_e], op=ALU.mult)
                    # ffn1-v + fx -> f1win rows re.. (slot = re-(r0-3) = slo+1)
                    for mt in range(2):
                        ps_f = p3ps.tile([128, NPX], F32, tag="ps_f")
                        nc.tensor.matmul(ps_f[:, 0:npx_e], wf1v[:, 0, mt], vn0[:, 0:npx_e], start=True, stop=False)
                        nc.tensor.matmul(ps_f[:, 0:npx_e], wf1v[:, 1, mt], vn1[:, 0:npx_e], start=False, stop=True)
                        fxs = p3.tile([128, NPX], BF16, tag="fxs")
                        nc.sync.dma_start(out=fxs[:, 0:npx_e], in_=fx_dram[mt, :, re * W:re * W + npx_e])
                        f1t = p3.tile([128, NPX], F32R, tag="f1t")
                        nc.vector.tensor_tensor(f1t[:, 0:npx_e], ps_f[:, 0:npx_e], fxs[:, 0:npx_e], op=ALU.add)
                        dstf = f1win[:, mt, slo + 1:slo + 1 + nrows, 1:1 + W]
                        nc.vector.tensor_copy(dstf,
                                              f1t[:, 0:npx_e].rearrange("p (a b) -> p a b", a=nrows))
                        if debug_taps:
                            f1c = p3.tile([128, NPX], F32, tag="f1c")
                            nc.vector.tensor_copy(f1c[:, 0:npx_e], f1t[:, 0:npx_e])
                            nc.sync.dma_start(out=dbg_f1[:, mt, re * W:re * W + npx_e], in_=f1c[:, 0:npx_e])

                def stage_b(rg, nrg, slg):
                    """dw conv rows rg..rg+nrg-1 (f1win slot of row rg = slg) + gelu + ffn2 -> out"""
                    npx_g = nrg * W
                    gsb = p3.tile([128, 2, NPX], F32R, tag="gsb")
                    for ct in range(2):
                        ps_g = p3ps.tile([128, NPX], F32, tag="ps_g")
                        for t_i, (dy, dx) in enumerate(TAPS):
                            sl0 = slg + dy
                            rhs = f1win[:, ct, sl0:sl0 + nrg, 1 + dx:1 + dx + W]
                            nc.tensor.matmul(ps_g[:, 0:npx_g], wdw[:, ct, t_i], rhs,
                                             start=(t_i == 0), stop=(t_i == 8))
                        nc.scalar.activation(gsb[:, ct, 0:npx_g], ps_g[:, 0:npx_g], AF.Gelu,
                                             bias=bias_g[:, ct])
                    for mt in range(2):
                        ps_out = p3ps.tile([128, NPX], F32, tag="ps_out")
                        nc.tensor.matmul(ps_out[:, 0:npx_g], wf2[:, 0, mt], gsb[:, 0, 0:npx_g], start=True, stop=False)
                        nc.tensor.matmul(ps_out[:, 0:npx_g], wf2[:, 1, mt], gsb[:, 1, 0:npx_g], start=False, stop=True)
                        osb = p3.tile([128, NPX], F32, tag="osb")
                        nc.scalar.copy(osb[:, 0:npx_g], ps_out[:, 0:npx_g])
                        nc.sync.dma_start(out=out_dram[mt, :, rg * W:rg * W + npx_g], in_=osb[:, 0:npx_g])

                for s_i in range(NSTRIP):
                    r0 = 16 * s_i
                    if s_i > 0:
                        nc.vector.tensor_copy(owin[:, 0:2], owin[:, 16:18])
                        nc.vector.tensor_copy(f1win[:, :, 0:3], f1win[:, :, 16:19])
                    # stage A over blocks
                    for b_i in range(NBLK):
                        rb = r0 + BLK_ROWS * b_i
                        dsto = owin[:, 4 * b_i + 2:4 * b_i + 6, 1:1 + W]
                        nc.vector.tensor_copy(dsto,
                                              vo_store[64:128, rb * W:(rb + 4) * W].rearrange("p (a b) -> p a b", a=4))
                        if s_i == 0 and b_i == 0:
                            stage_a(0, 3, 2, b_i, s_i)
                        else:
                            stage_a(rb - 1, 4, 4 * b_i + 1, b_i, s_i)
                    if s_i == NSTRIP - 1:
                        # f1 row 127 epilogue (o rows 126..128 ; owin slot of row 126 = 16)
                        stage_a(127, 1, 17, 0, s_i)
                    # stage B over blocks (rows r0-2 .. r0+13)
                    for b_i in range(NBLK):
                        rb = r0 + BLK_ROWS * b_i
                        if s_i == 0 and b_i == 0:
                            stage_b(0, 2, 3)
                        else:
                            stage_b(rb - 2, 4, 4 * b_i + 1)
                # out rows 126,127 (f1win slot of row 126 = 126-109 = 17 ; slot19 zero? need row 128->slot19)
                stage_b(126, 2, 17)
    return nc


def _prep_host(inputs):
    f32 = np.float32
    w_cq = np.asarray(inputs["w_cq"], f32)
    w_ckv = np.asarray(inputs["w_ckv"], f32)
    ln_q_w = np.asarray(inputs["ln_q_w"], f32); ln_q_b = np.asarray(inputs["ln_q_b"], f32)
    ln_kv_w = np.asarray(inputs["ln_kv_w"], f32); ln_kv_b = np.asarray(inputs["ln_kv_b"], f32)
    w_kv = np.asarray(inputs["w_kv"], f32)
    w_kvdw = np.asarray(inputs["w_kvdw"], f32)
    w_q = np.asarray(inputs["w_q"], f32)
    temperature = np.asarray(inputs["temperature"], f32)
    w_po = np.asarray(inputs["w_po"], f32)
    w_expand = np.asarray(inputs["w_expand"], f32)
    ln_out_w = np.asarray(inputs["ln_out_w"], f32); ln_out_b = np.asarray(inputs["ln_out_b"], f32)
    w_ffn1 = np.asarray(inputs["w_ffn1"], f32)
    w_ffn_dw = np.asarray(inputs["w_ffn_dw"], f32)
    w_ffn2 = np.asarray(inputs["w_ffn2"], f32)

    d = {}
    def conv_lhsT(wc):
        a = np.zeros((128, 2, 9, S + 1), f32)
        for kt in range(2):
            blk = wc[:, kt * 128:(kt + 1) * 128]           # [S, 128, 3, 3]
            a[:, kt, :, :S] = blk.transpose(1, 2, 3, 0).reshape(128, 9, S)
            a[:, kt, :, S] = blk.sum(axis=0).reshape(128, 9)
        return a
    d["wcq"] = conv_lhsT(w_cq)
    d["wckv"] = conv_lhsT(w_ckv)
    w_q_eff = w_q * ln_q_w[None, :, None, None]
    d["bias_q"] = (w_q * ln_q_b[None, :, None, None]).sum(axis=(1, 2, 3)).reshape(S, 1)
    wqkv = np.zeros((128, 9, 128), f32)
    wqkv[0:64, :, 0:S] = w_q_eff.transpose(1, 2, 3, 0).reshape(S, 9, S)
    w_kv_g = w_kv[:, :, 0, 0] * ln_kv_w[None, :]
    w_kv_eff = w_kvdw[:, 0][:, None] * w_kv_g[:, :, None, None]   # [2S, S, 3, 3]
    d["bias_kv"] = (w_kvdw[:, 0].sum(axis=(1, 2)) * (w_kv[:, :, 0, 0] @ ln_kv_b)).reshape(2 * S, 1)
    wqkv[64:128, :, :] = w_kv_eff.transpose(1, 2, 3, 0).reshape(S, 9, 2 * S)
    d["wqkv"] = wqkv
    d["wpo"] = np.ascontiguousarray(w_po[:, :, 0, 0].T)
    d["wexp"] = np.ascontiguousarray(
        w_expand.transpose(1, 2, 3, 0).reshape(S, 9, C)).astype(ml_dtypes.bfloat16)
    w1 = w_ffn1[:, :, 0, 0]
    w1x = w1[:, :C]
    w1v = w1[:, C:] * ln_out_w[None, :]
    def one_by_one_lhsT(wm):
        a = np.zeros((128, 2, 2, 128), f32)
        for kt in range(2):
            for mt in range(2):
                a[:, kt, mt, :] = wm[mt * 128:(mt + 1) * 128, kt * 128:(kt + 1) * 128].T
        return a
    d["wf1x"] = one_by_one_lhsT(w1x)
    d["wf1v"] = one_by_one_lhsT(w1v)
    bias_f1 = w1[:, C:] @ ln_out_b
    dw_t = w_ffn_dw[:, 0].reshape(C, 9)
    d["bias_g"] = np.ascontiguousarray(
        (bias_f1 * dw_t.sum(1)).reshape(2, 128, 1).transpose(1, 0, 2))
    wdw = np.zeros((128, 2, 9, 128), f32)
    for ct in range(2):
        for t in range(9):
            np.fill_diagonal(wdw[:, ct, t, :], dw_t[ct * 128:(ct + 1) * 128, t])
    d["wdw"] = wdw
    d["wf2"] = one_by_one_lhsT(w_ffn2[:, :, 0, 0])
    stat_cq = np.zeros((S + 1, 1), f32)
    stat_cq[:S, 0] = 1.0 / S
    stat_cq[S, 0] = -1.0 / (S * S)
    d["stat_cq"] = stat_cq
    bc2a = np.zeros((1, 128), f32); bc2a[0, 0:64] = 1.0
    bc2b = np.zeros((1, 128), f32); bc2b[0, 64:128] = 1.0
    d["bc2a"] = bc2a; d["bc2b"] = bc2b
    d["ones128"] = np.ones((1, 128), f32)
    stat256 = np.zeros((128, 2), f32)
    stat256[:, 0] = 1.0 / C
    stat256[:, 1] = 1.0 / C
    d["stat256"] = stat256
    d["tempv"] = np.repeat(temperature.reshape(HEADS), S // HEADS).reshape(S, 1).astype(f32)
    mask = np.zeros((S, S), f32)
    for h in range(HEADS):
        mask[h * 8:(h + 1) * 8, h * 8:(h + 1) * 8] = 1.0
    d["maskbd"] = mask
    d["ident"] = np.eye(128, dtype=f32).astype(ml_dtypes.bfloat16)
    d["identr"] = np.eye(S, dtype=f32)
    d["onesr"] = np.ones((1, S), f32)
    d["zeros"] = np.zeros((128, 2600), f32)
    d["zerosb"] = np.zeros((128, 2600), f32).astype(ml_dtypes.bfloat16)
    return d


def _pad_input(x):
    """[C,H,W] f32 -> [128, 2, H+4, WP] zero-padded, partition-major"""
    out = np.zeros((128, 2, H + 4, WP), np.float32)
    out[:, :, 2:H + 2, 1:W + 1] = x.reshape(2, 128, H, W).transpose(1, 0, 2, 3)
    return out


def kernel(**inputs):
    key = "nc"
    if key not in _CACHED:
        nc = build_nc(debug_taps=False)
        nc.finalize()
        _CACHED[key] = nc
    nc = _CACHED[key]
    d = _prep_host(inputs)
    x = np.asarray(inputs["x"], np.float32)
    y = np.asarray(inputs["y"], np.float32)
    in_maps = []
    for i in range(B):
        m = dict(d)
        m["x"] = _pad_input(x[i])
        m["y"] = _pad_input(y[i])
        in_maps.append(m)
    res = run_bass_kernel_spmd(nc, in_maps, list(range(B)))
    out = np.stack([res.results[i]["out"].reshape(C, H, W) for i in range(B)])
    return out.astype(np.float32)



# revision 42
# speedup vs baseline: 1.0179x; 1.0179x over previous
# Trainium2 Bass kernel for ChannelAttentionBlock (B=8,C=256,H=W=128,S=64,HEADS=8)
# Data-parallel over batch: 1 sample per NeuronCore, 8 cores.
#
# v2: fp8 DoubleRow for the q-side convs (scale-invariant attention path),
# dy-stacked kv/expand convs, wpo folded into expand, strip-batched LN sqrt
# (keeps the Act table stable), bf16 FFN tail, engine rebalance.
import contextlib
import numpy as np
import ml_dtypes

import concourse.bass as bass
from concourse import bacc
import concourse.mybir as mybir
from concourse.bass_utils import run_bass_kernel_spmd
from concourse.tile import TileContext

F32R = mybir.dt.float32r
F32 = mybir.dt.float32
BF16 = mybir.dt.bfloat16
FP8 = mybir.dt.float8e4
F8NP = mybir.dt.np(mybir.dt.float8e4)
AF = mybir.ActivationFunctionType
ALU = mybir.AluOpType
DRMODE = mybir.MatmulPerfMode.DoubleRow

B, C, H, W = 8, 256, 128, 128
S = 64
HEADS = 8
HW = H * W
WP = W + 2          # padded row width
NSTRIP = H // 16
BLK_ROWS = 4
NBLK = 4
NPX = BLK_ROWS * W  # 512
EPS = 1e-5
SC8 = 64.0          # fp8 weight scale (washes out in LN / q-normalization)
EPS_CQ = EPS * SC8 * SC8

TAPS = [(dy, dx) for dy in (-1, 0, 1) for dx in (-1, 0, 1)]

_CACHED = {}


def _ins_dim(ap, entry):
    """Insert a [stride, count] dim right after the partition dim of an AP."""
    dims = [list(d) for d in ap.ap]
    return bass.AP(tensor=ap.tensor, offset=ap.offset,
                   ap=[dims[0], list(entry)] + dims[1:])


def build_nc():
    nc = bacc.Bacc("TRN2", target_bir_lowering=False, debug=False)

    # ------------- DRAM parameters (host layouts) -------------
    x_in = nc.dram_tensor("x", [128, 2, H + 4, WP], F32R, kind="ExternalInput")
    x8_in = nc.dram_tensor("x8", [128, 2, H + 4, WP], FP8, kind="ExternalInput")
    y_in = nc.dram_tensor("y", [128, 2, H + 4, WP], F32R, kind="ExternalInput")
    wcq8_in = nc.dram_tensor("wcq8", [128, 9, 2, 128], FP8, kind="ExternalInput")
    wckv_in = nc.dram_tensor("wckv", [128, 2, 9, S + 1], F32R, kind="ExternalInput")
    wq2_in = nc.dram_tensor("wq2", [128, 3, 2, S], FP8, kind="ExternalInput")
    wkva_in = nc.dram_tensor("wkva", [128, 3, 128], F32R, kind="ExternalInput")
    wkvb_in = nc.dram_tensor("wkvb", [S, 3, 128], F32R, kind="ExternalInput")
    wexpa_in = nc.dram_tensor("wexpa", [128, 3, C], BF16, kind="ExternalInput")
    wexpb_in = nc.dram_tensor("wexpb", [S, 3, C], BF16, kind="ExternalInput")
    wf1x_in = nc.dram_tensor("wf1x", [128, 2, 2, 128], F32R, kind="ExternalInput")
    wf1v_in = nc.dram_tensor("wf1v", [128, 2, 2, 128], F32R, kind="ExternalInput")
    wdw_in = nc.dram_tensor("wdw", [128, 2, 9, 128], BF16, kind="ExternalInput")
    wf2_in = nc.dram_tensor("wf2", [128, 2, 2, 128], BF16, kind="ExternalInput")
    stat_cq_in = nc.dram_tensor("stat_cq", [S + 1, 1], F32R, kind="ExternalInput")
    bc2_in = nc.dram_tensor("bc2", [33, 128], F32R, kind="ExternalInput")
    ones128_in = nc.dram_tensor("ones128", [1, 128], F32R, kind="ExternalInput")
    stat256_in = nc.dram_tensor("stat256", [128, 2], F32R, kind="ExternalInput")
    bias_q_in = nc.dram_tensor("bias_q", [S, 1], F32, kind="ExternalInput")
    bias_kv_in = nc.dram_tensor("bias_kv", [128, 1], F32, kind="ExternalInput")
    bias_g_in = nc.dram_tensor("bias_g", [128, 2, 1], F32, kind="ExternalInput")
    temp_in = nc.dram_tensor("tempv", [S, 1], F32, kind="ExternalInput")
    mask_in = nc.dram_tensor("maskbd", [S, S], F32R, kind="ExternalInput")
    identr_in = nc.dram_tensor("identr", [S, S], F32R, kind="ExternalInput")
    ones_in = nc.dram_tensor("onesr", [1, S], F32R, kind="ExternalInput")
    zeros8_in = nc.dram_tensor("zeros8", [128, 2600], FP8, kind="ExternalInput")
    zeros_in = nc.dram_tensor("zeros", [128, 2600], F32R, kind="ExternalInput")
    zerosb_in = nc.dram_tensor("zerosb", [128, 2600], BF16, kind="ExternalInput")

    fx_dram = nc.dram_tensor("fx_dram", [2, 128, HW], BF16)
    out_dram = nc.dram_tensor("out", [2, 128, HW], F32, kind="ExternalOutput")

    with TileContext(nc) as tc:
        with tc.tile_pool(name="persist", bufs=1) as persist:
            qk_store = persist.tile([128, HW], BF16, tag="qk_store")
            qkT = persist.tile([128, H, 128], BF16, tag="qkT")
            vo_store = persist.tile([128, HW], BF16, tag="vo_store")
            rq2 = persist.tile([S, 1], F32, tag="rq2")
            rk2 = persist.tile([S, 1], F32, tag="rk2")
            nc.vector.memset(rq2, 0.0)
            nc.vector.memset(rk2, 0.0)

            # ================= PHASE 1 =================
            with tc.tile_pool(name="p1w", bufs=1) as p1w, \
                 tc.tile_pool(name="p1", bufs=2) as p1, \
                 tc.tile_pool(name="p1ps", bufs=1, space="PSUM") as p1ps:
                wcq8 = p1w.tile([128, 9, 2, 128], FP8, tag="wcq8")
                nc.sync.dma_start(out=wcq8, in_=wcq8_in[:, :, :, :])
                wckv = p1w.tile([128, 2, 9, S + 1], F32R, tag="wckv")
                nc.sync.dma_start(out=wckv, in_=wckv_in[:, :, :, :])
                wq2 = p1w.tile([128, 3, 2, S], FP8, tag="wq2")
                nc.sync.dma_start(out=wq2, in_=wq2_in[:, :, :, :])
                wkva = p1w.tile([128, 3, 128], F32R, tag="wkva")
                nc.sync.dma_start(out=wkva, in_=wkva_in[:, :, :])
                wkvb = p1w.tile([S, 3, 128], F32R, tag="wkvb")
                nc.sync.dma_start(out=wkvb, in_=wkvb_in[:, :, :])
                wf1x = p1w.tile([128, 2, 2, 128], F32R, tag="wf1x")
                nc.sync.dma_start(out=wf1x, in_=wf1x_in[:, :, :, :])
                stat_cq = p1w.tile([S + 1, 1], F32R, tag="stat_cq")
                nc.sync.dma_start(out=stat_cq, in_=stat_cq_in[:, :])
                bc2 = p1w.tile([33, 128], F32R, tag="bc2")
                nc.sync.dma_start(out=bc2, in_=bc2_in[:, :])
                # broadcast rhs: row 0 = rsqrt (Act), row 32 = channel sum (DVE);
                # rows 1..31 stay zero (quad-aligned partition starts only)
                brs = p1w.tile([33, NPX], F32R, tag="brs")
                nc.sync.dma_start(out=brs, in_=zeros_in[0:33, 0:NPX])
                bias_q = p1w.tile([S, 1], F32, tag="bias_q")
                nc.sync.dma_start(out=bias_q, in_=bias_q_in[:, :])
                bias_kv = p1w.tile([128, 1], F32, tag="bias_kv")
                nc.sync.dma_start(out=bias_kv, in_=bias_kv_in[:, :])

                xwin = p1w.tile([128, 2, 18, WP], F32R, tag="xwin")
                xwin8 = p1w.tile([128, 2, 18, WP], FP8, tag="xwin8")
                ywin = p1w.tile([128, 2, 18, WP], F32R, tag="ywin")
                # nq2: LN'd xq (fp8); partitions 0:64 = row(slot), 64:128 = row(slot)+1
                nq2 = p1w.tile([128, 19, WP], FP8, tag="nq2")
                nc.sync.dma_start(out=nq2.rearrange("p a b -> p (a b)"),
                                  in_=zeros8_in[:, :19 * WP])
                # nkv: LN'd ykv; partitions 0:64 = row(slot), 64:128 = row(slot)+1
                nkv = p1w.tile([128, 19, WP], F32R, tag="nkv")
                nc.sync.dma_start(out=nkv.rearrange("p a b -> p (a b)"),
                                  in_=zeros_in[:, :19 * WP])

                def ln_tail(ps_c, eps, t_on_act, write_out):
                    """LN stats+apply for conv PSUM ps_c [S+1, NPX]."""
                    t_sb = p1.tile([S, NPX], F32R, tag="t_sb")
                    if t_on_act:
                        nc.scalar.copy(t_sb, ps_c[0:S])
                    else:
                        nc.vector.tensor_copy(t_sb, ps_c[0:S])
                    sq_sb = p1.tile([S + 1, NPX], F32R, tag="sq_sb")
                    nc.scalar.activation(sq_sb, ps_c[0:S + 1], AF.Square)
                    ps_v = p1ps.tile([1, NPX], F32, tag="small")
                    nc.tensor.matmul(ps_v, stat_cq, sq_sb, start=True, stop=True)
                    varr = p1.tile([1, NPX], F32, tag="varr", bufs=1)
                    nc.vector.tensor_scalar_add(varr, ps_v[0:1], eps)
                    rcpv = p1.tile([1, NPX], F32, tag="rcpv", bufs=1)
                    nc.vector.reciprocal_approx_fast(out=rcpv, in_=varr)
                    nc.scalar.activation(brs[0:1], rcpv, AF.Sqrt)
                    nc.vector.tensor_copy(brs[32:33], ps_c[S:S + 1])
                    ps_b = p1ps.tile([128, NPX], F32, tag="bc", bufs=2)
                    nc.tensor.matmul(ps_b, bc2, brs, start=True, stop=True)
                    d_sb = p1.tile([S, NPX], F32, tag="d_sb")
                    nc.vector.tensor_tensor(d_sb, t_sb, ps_b[0:S], op=ALU.subtract)
                    write_out(d_sb, ps_b)

                for s_i in range(NSTRIP):
                    r0 = 16 * s_i
                    if s_i > 0:
                        # x/y window carries were issued early (during block 3
                        # of the previous strip); only the LN-output windows
                        # must wait for the previous strip's last LN writes.
                        nc.gpsimd.tensor_copy(nq2[:, 0:2], nq2[:, 16:18])
                        nc.vector.tensor_copy(nkv[:, 0:2], nkv[:, 16:18])
                    for b_i in range(NBLK):
                        rb = r0 + BLK_ROWS * b_i
                        # xwin slot i = row (r0-1)+i
                        if s_i == 0 and b_i == 0:
                            nc.sync.dma_start(out=xwin[:, :, 0:6], in_=x_in[:, :, 1:7])
                            nc.sync.dma_start(out=xwin8[:, :, 0:6], in_=x8_in[:, :, 1:7])
                            nc.sync.dma_start(out=ywin[:, :, 0:6], in_=y_in[:, :, 1:7])
                        else:
                            sl = 4 * b_i + 2
                            nc.sync.dma_start(out=xwin[:, :, sl:sl + 4], in_=x_in[:, :, rb + 3:rb + 7])
                            nc.sync.dma_start(out=xwin8[:, :, sl:sl + 4], in_=x8_in[:, :, rb + 3:rb + 7])
                            nc.sync.dma_start(out=ywin[:, :, sl:sl + 4], in_=y_in[:, :, rb + 3:rb + 7])
                        if b_i == 3 and s_i < NSTRIP - 1:
                            # early carry for next strip: sources are DMA-loaded
                            # slots 16:18, dests were last read in block 0/1.
                            nc.vector.tensor_copy(xwin[:, :, 0:2], xwin[:, :, 16:18])
                            nc.gpsimd.tensor_copy(xwin8[:, :, 0:2], xwin8[:, :, 16:18])
                            nc.vector.tensor_copy(ywin[:, :, 0:2], ywin[:, :, 16:18])

                        # ---- cq conv (fp8 DoubleRow, 9 taps)
                        ps_cq = p1ps.tile([128, NPX], F32, tag="ps_cq")
                        for t_i, (dy, dx) in enumerate(TAPS):
                            sl0 = 4 * b_i + 1 + dy
                            rhs = xwin8[:, :, sl0:sl0 + 4, 1 + dx:1 + dx + W]
                            nc.tensor.matmul(ps_cq, wcq8[:, t_i], rhs,
                                             start=(t_i == 0), stop=(t_i == 8),
                                             perf_mode=DRMODE)
                        # ---- ckv conv (f32r, 18 matmuls)
                        ps_ckv = p1ps.tile([S + 1, NPX], F32, tag="ps_ckv")
                        first = True
                        for kt in range(2):
                            for t_i, (dy, dx) in enumerate(TAPS):
                                sl0 = 4 * b_i + 1 + dy
                                rhs = ywin[:, kt, sl0:sl0 + 4, 1 + dx:1 + dx + W]
                                nc.tensor.matmul(ps_ckv, wckv[:, kt, t_i], rhs,
                                                 start=first, stop=(kt == 1 and t_i == 8))
                                first = False
                        # ---- fx (ffn1 x-half) rows rb..rb+3
                        for mt in range(2):
                            ps_fx = p1ps.tile([128, NPX], F32, tag="ps_fx")
                            for kt in range(2):
                                rhs = xwin[:, kt, 4 * b_i + 1:4 * b_i + 5, 1:1 + W]
                                nc.tensor.matmul(ps_fx, wf1x[:, kt, mt], rhs,
                                                 start=(kt == 0), stop=(kt == 1))
                            fx_sb = p1.tile([128, NPX], BF16, tag="fx_sb")
                            nc.scalar.copy(fx_sb, ps_fx)
                            nc.sync.dma_start(out=fx_dram[mt, :, rb * W:(rb + 4) * W], in_=fx_sb)

                        # ---- LN tails -> nwin8 (xq, fp8) / nkv (ykv, f32r)
                        sl_w = 4 * b_i + 2   # write slot of row rb

                        def write_xq(d_sb, ps_b, sl_w=sl_w):
                            dst = nq2[0:S, sl_w:sl_w + 4, 1:1 + W]
                            nc.vector.tensor_tensor(
                                dst, d_sb.rearrange("p (a b) -> p a b", a=4),
                                ps_b[64:128].rearrange("p (a b) -> p a b", a=4),
                                op=ALU.mult)
                            nc.gpsimd.tensor_copy(nq2[S:128, sl_w - 1:sl_w + 3, 1:1 + W],
                                                  nq2[0:S, sl_w:sl_w + 4, 1:1 + W])

                        def write_kv(d_sb, ps_b, sl_w=sl_w):
                            dst = nkv[0:S, sl_w:sl_w + 4, 1:1 + W]
                            nc.vector.tensor_tensor(
                                dst, d_sb.rearrange("p (a b) -> p a b", a=4),
                                ps_b[64:128].rearrange("p (a b) -> p a b", a=4),
                                op=ALU.mult)
                            # duplicate rows one slot earlier in partitions 64:128
                            nc.gpsimd.tensor_copy(nkv[S:128, sl_w - 1:sl_w + 3, 1:1 + W],
                                                  nkv[0:S, sl_w:sl_w + 4, 1:1 + W])

                        ln_tail(ps_cq, EPS_CQ, True, write_xq)
                        ln_tail(ps_ckv, EPS, False, write_kv)

                        # ---- q / kv convs (lag 1 row)
                        if s_i == 0 and b_i == 0:
                            rq, nrows, sl_base = 0, 3, 2
                        else:
                            rq, nrows, sl_base = rb - 1, 4, 4 * b_i + 1
                        npx_q = nrows * W
                        s_m1 = sl_base - 1
                        s_p1 = sl_base + 1
                        # q: fp8; 3 K=128 DR matmuls (low/high rows x 2 k-tiles
                        # cover all three dy taps per dx)
                        ps_q = p1ps.tile([S + 1, NPX], F32, tag="ps_q")
                        for dxi in range(3):
                            base = nq2[:, s_m1:s_m1 + nrows, dxi:dxi + W]
                            rhs = _ins_dim(base, [WP, 2])
                            nc.tensor.matmul(ps_q[0:S, 0:npx_q], wq2[:, dxi], rhs,
                                             start=(dxi == 0), stop=(dxi == 2),
                                             perf_mode=DRMODE)
                        # kv: f32r; 3 dy-pair streams (K=128) + 3 singles (K=64)
                        ps_kv = p1ps.tile([128, NPX], F32, tag="ps_kv")
                        for dxi in range(3):
                            rhs = nkv[:, s_m1:s_m1 + nrows, dxi:dxi + W]
                            nc.tensor.matmul(ps_kv[:, 0:npx_q], wkva[:, dxi], rhs,
                                             start=(dxi == 0), stop=False)
                        for dxi in range(3):
                            rhs = nkv[0:S, s_p1:s_p1 + nrows, dxi:dxi + W]
                            nc.tensor.matmul(ps_kv[:, 0:npx_q], wkvb[:, dxi], rhs,
                                             start=False, stop=(dxi == 2))
                        # bias + store bf16
                        c0 = rq * W
                        nc.scalar.activation(qk_store[0:S, c0:c0 + npx_q], ps_q[0:S, 0:npx_q],
                                             AF.Identity, bias=bias_q)
                        nc.scalar.activation(qk_store[S:128, c0:c0 + npx_q], ps_kv[0:S, 0:npx_q],
                                             AF.Identity, bias=bias_kv[0:S])
                        nc.scalar.activation(vo_store[0:S, c0:c0 + npx_q], ps_kv[S:128, 0:npx_q],
                                             AF.Identity, bias=bias_kv[S:128])
                        nc.scalar.dma_start_transpose(
                            out=qkT[:, rq:rq + nrows, :],
                            in_=qk_store[:, c0:c0 + npx_q])
                        # channel norms (accumulated)
                        scr = p1.tile([S, NPX], BF16, tag="scr")
                        q_acc = p1.tile([S, 1], F32, tag="q_acc")
                        nc.scalar.activation(scr[:, 0:npx_q], qk_store[0:S, c0:c0 + npx_q],
                                             AF.Square, accum_out=q_acc)
                        nc.vector.tensor_tensor(rq2, rq2, q_acc, op=ALU.add)
                        scr2 = p1.tile([S, NPX], BF16, tag="scr2")
                        k_acc = p1.tile([S, 1], F32, tag="k_acc")
                        nc.scalar.activation(scr2[:, 0:npx_q], qk_store[S:128, c0:c0 + npx_q],
                                             AF.Square, accum_out=k_acc)
                        nc.vector.tensor_tensor(rk2, rk2, k_acc, op=ALU.add)

                # epilogue: q/kv row 127 (slot of row 127 = 17; slot 18 zero)
                rq, nrows, sl_base = 127, 1, 17
                npx_q = nrows * W
                s_m1, s_p1 = sl_base - 1, sl_base + 1
                ps_q = p1ps.tile([S + 1, NPX], F32, tag="ps_q")
                for dxi in range(3):
                    base = nq2[:, s_m1:s_m1 + nrows, dxi:dxi + W]
                    rhs = _ins_dim(base, [WP, 2])
                    nc.tensor.matmul(ps_q[0:S, 0:npx_q], wq2[:, dxi], rhs,
                                     start=(dxi == 0), stop=(dxi == 2), perf_mode=DRMODE)
                ps_kv = p1ps.tile([128, NPX], F32, tag="ps_kv")
                for dxi in range(3):
                    rhs = nkv[:, s_m1:s_m1 + nrows, dxi:dxi + W]
                    nc.tensor.matmul(ps_kv[:, 0:npx_q], wkva[:, dxi], rhs,
                                     start=(dxi == 0), stop=False)
                for dxi in range(3):
                    rhs = nkv[0:S, s_p1:s_p1 + nrows, dxi:dxi + W]
                    nc.tensor.matmul(ps_kv[:, 0:npx_q], wkvb[:, dxi], rhs,
                                     start=False, stop=(dxi == 2))
                c0 = rq * W
                nc.scalar.activation(qk_store[0:S, c0:c0 + npx_q], ps_q[0:S, 0:npx_q],
                                     AF.Identity, bias=bias_q)
                nc.scalar.activation(qk_store[S:128, c0:c0 + npx_q], ps_kv[0:S, 0:npx_q],
                                     AF.Identity, bias=bias_kv[0:S])
                nc.scalar.activation(vo_store[0:S, c0:c0 + npx_q], ps_kv[S:128, 0:npx_q],
                                     AF.Identity, bias=bias_kv[S:128])
                nc.scalar.dma_start_transpose(out=qkT[:, rq:rq + 1, :],
                                              in_=qk_store[:, c0:c0 + npx_q])
                scr = p1.tile([S, NPX], BF16, tag="scr")
                q_acc = p1.tile([S, 1], F32, tag="q_acc")
                nc.scalar.activation(scr[:, 0:npx_q], qk_store[0:S, c0:c0 + npx_q],
                                     AF.Square, accum_out=q_acc)
                nc.vector.tensor_tensor(rq2, rq2, q_acc, op=ALU.add)
                scr2 = p1.tile([S, NPX], BF16, tag="scr2")
                k_acc = p1.tile([S, 1], F32, tag="k_acc")
                nc.scalar.activation(scr2[:, 0:npx_q], qk_store[S:128, c0:c0 + npx_q],
                                     AF.Square, accum_out=k_acc)
                nc.vector.tensor_tensor(rk2, rk2, k_acc, op=ALU.add)

            # -------- phase-3 weights prefetch (overlaps phase 2) --------
            p3w_stack = contextlib.ExitStack()
            p3w = p3w_stack.enter_context(tc.tile_pool(name="p3w", bufs=1))
            wexpa = p3w.tile([128, 3, C], BF16, tag="wexpa")
            nc.sync.dma_start(out=wexpa, in_=wexpa_in[:, :, :])
            wexpb = p3w.tile([S, 3, C], BF16, tag="wexpb")
            nc.sync.dma_start(out=wexpb, in_=wexpb_in[:, :, :])
            wf1v = p3w.tile([128, 2, 2, 128], F32R, tag="wf1v")
            nc.sync.dma_start(out=wf1v, in_=wf1v_in[:, :, :, :])
            wdw = p3w.tile([128, 2, 9, 128], BF16, tag="wdw")
            nc.sync.dma_start(out=wdw, in_=wdw_in[:, :, :, :])
            wf2 = p3w.tile([128, 2, 2, 128], BF16, tag="wf2")
            nc.sync.dma_start(out=wf2, in_=wf2_in[:, :, :, :])
            stat256 = p3w.tile([128, 2], F32R, tag="stat256")
            nc.sync.dma_start(out=stat256, in_=stat256_in[:, :])
            ones128 = p3w.tile([1, 128], F32R, tag="ones128")
            nc.sync.dma_start(out=ones128, in_=ones128_in[:, :])
            bias_g = p3w.tile([128, 2, 1], F32, tag="bias_g")
            nc.sync.dma_start(out=bias_g, in_=bias_g_in[:, :, :])
            # owin2: o rows; partitions 0:64 = row(slot), 64:128 = row(slot)+1
            owin2 = p3w.tile([128, 19, WP], BF16, tag="owin2")
            nc.sync.dma_start(out=owin2.rearrange("p a b -> p (a b)"),
                              in_=zerosb_in[:, :19 * WP])
            # f1win: slot i = row (r0-3)+i ; slot 19 always zero
            f1win = p3w.tile([128, 2, 20, WP], BF16, tag="f1win")
            for _h in range(2):
                nc.sync.dma_start(out=f1win[:, _h].rearrange("p a b -> p (a b)"),
                                  in_=zerosb_in[:, :20 * WP])
            t_strip = p3w.tile([128, 2, 5, NPX], F32R, tag="t_strip")

            # ================= PHASE 2: attention =================
            with tc.tile_pool(name="p2", bufs=2) as p2, \
                 tc.tile_pool(name="p2one", bufs=1) as p2one, \
                 tc.tile_pool(name="p2ps", bufs=2, space="PSUM") as p2ps:
                NACC = 4
                gs = []
                for j in range(NACC):
                    g_acc = p2ps.tile([S, S], F32, tag=f"g{j}", bufs=1, name=f"g_acc{j}")
                    gs.append(g_acc)
                for tb in range(H):
                    j = tb % NACC
                    nc.tensor.matmul(gs[j], qkT[:, tb, 0:S], qkT[:, tb, S:128],
                                     start=(tb < NACC), stop=(tb >= H - NACC))
                g_sb = p2one.tile([S, S], F32, tag="g_sb")
                nc.scalar.copy(g_sb, gs[0])
                for j in range(1, NACC):
                    nc.vector.tensor_tensor(g_sb, g_sb, gs[j], op=ALU.add)
                rqs = p2one.tile([S, 1], F32, tag="rqs")
                rks = p2one.tile([S, 1], F32, tag="rks")
                sq1 = p2one.tile([S, 1], F32, tag="sq1")
                sq2 = p2one.tile([S, 1], F32, tag="sq2")
                nc.vector.reciprocal_approx_fast(out=sq1, in_=rq2)
                nc.scalar.activation(rqs, sq1, AF.Sqrt)
                nc.vector.reciprocal_approx_fast(out=sq2, in_=rk2)
                nc.scalar.activation(rks, sq2, AF.Sqrt)
                temp_t = p2one.tile([S, 1], F32, tag="temp_t")
                nc.sync.dma_start(out=temp_t, in_=temp_in[:, :])
                nc.vector.tensor_tensor(rqs, rqs, temp_t, op=ALU.mult)
                nc.vector.tensor_scalar_mul(g_sb, g_sb, rqs)
                rk_row = p2one.tile([1, S], F32R, tag="rk_row")
                nc.sync.dma_start(out=rk_row, in_=rks[:, :].bitcast(F32R))
                ones1 = p2one.tile([1, S], F32R, tag="ones1")
                nc.sync.dma_start(out=ones1, in_=ones_in[:, :])
                rkb_ps = p2ps.tile([S, S], F32, tag="rkb_ps", bufs=1)
                nc.tensor.matmul(rkb_ps, ones1, rk_row, start=True, stop=True)
                s_sb = p2one.tile([S, 8, 8], F32, tag="s_sb")
                nc.vector.tensor_tensor(s_sb.rearrange("p a b -> p (a b)"), g_sb, rkb_ps, op=ALU.mult)
                mx = p2one.tile([S, 8], F32, tag="mx")
                nc.vector.reduce_max(mx, s_sb, axis=mybir.AxisListType.X)
                mxb = bass.AP(tensor=mx.tensor, offset=mx.offset,
                              ap=[list(mx.ap[0]), list(mx.ap[1]), [0, 8]])
                e_sb = p2one.tile([S, 8, 8], F32, tag="e_sb")
                nc.vector.tensor_tensor(e_sb, s_sb, mxb, op=ALU.subtract)
                ex_sb = p2one.tile([S, 8, 8], F32, tag="ex_sb")
                nc.scalar.activation(ex_sb, e_sb, AF.Exp)
                sm = p2one.tile([S, 8], F32, tag="sm")
                nc.vector.reduce_sum(sm, ex_sb, axis=mybir.AxisListType.X)
                rs = p2one.tile([S, 8], F32, tag="rs")
                nc.vector.reciprocal_approx_fast(out=rs, in_=sm)
                rsb = bass.AP(tensor=rs.tensor, offset=rs.offset,
                              ap=[list(rs.ap[0]), list(rs.ap[1]), [0, 8]])
                attn = p2one.tile([S, S], F32R, tag="attn")
                nc.vector.tensor_tensor(attn.rearrange("p (a b) -> p a b", a=8), ex_sb, rsb, op=ALU.mult)
                maskbd = p2one.tile([S, S], F32R, tag="maskbd")
                nc.sync.dma_start(out=maskbd, in_=mask_in[:, :])
                attn_m = p2one.tile([S, S], F32R, tag="attn_m")
                nc.vector.tensor_tensor(attn_m, attn, maskbd, op=ALU.mult)
                identr = p2one.tile([S, S], F32R, tag="identr")
                nc.sync.dma_start(out=identr, in_=identr_in[:, :])
                attn_tp = p2ps.tile([S, S], F32R, tag="attn_tp", bufs=1)
                nc.tensor.transpose(attn_tp, attn_m, identr)
                attn_t = p2one.tile([S, S], BF16, tag="attn_t")
                nc.scalar.copy(attn_t, attn_tp)
                for blk in range(HW // NPX):
                    ps_o = p2ps.tile([S, NPX], F32, tag="ps_o")
                    nc.tensor.matmul(ps_o, attn_t, vo_store[0:S, blk * NPX:(blk + 1) * NPX],
                                     start=True, stop=True)
                    nc.vector.tensor_copy(vo_store[S:128, blk * NPX:(blk + 1) * NPX], ps_o)

            # ================= PHASE 3: expand(+po) + LN + FFN =================
            with tc.tile_pool(name="p3", bufs=2) as p3, \
                 tc.tile_pool(name="p3ps", bufs=1, space="PSUM") as p3ps:

                def stage_a_pre(bslot, re, nrows, slo):
                    npx_e = nrows * W
                    s_m1, s_p1 = slo - 1, slo + 1
                    ps_e0 = p3ps.tile([128, NPX], F32, tag="ps_e0")
                    ps_e1 = p3ps.tile([128, NPX], F32, tag="ps_e1")
                    for dxi in range(3):
                        rhs_p = owin2[:, s_m1:s_m1 + nrows, dxi:dxi + W]
                        rhs_s = owin2[0:S, s_p1:s_p1 + nrows, dxi:dxi + W]
                        for mt, ps_e in ((0, ps_e0), (1, ps_e1)):
                            nc.tensor.matmul(ps_e[:, 0:npx_e],
                                             wexpa[:, dxi, mt * 128:(mt + 1) * 128], rhs_p,
                                             start=(dxi == 0), stop=False)
                            nc.tensor.matmul(ps_e[:, 0:npx_e],
                                             wexpb[:, dxi, mt * 128:(mt + 1) * 128], rhs_s,
                                             start=False, stop=(dxi == 2 and mt == 1))
                    nc.scalar.copy(t_strip[:, 0, bslot, 0:npx_e], ps_e0[:, 0:npx_e])
                    nc.scalar.copy(t_strip[:, 1, bslot, 0:npx_e], ps_e1[:, 0:npx_e])
                    sq0 = p3.tile([128, NPX], F32R, tag="sq0")
                    sq1t = p3.tile([128, NPX], F32R, tag="sq1t")
                    nc.scalar.activation(sq0[:, 0:npx_e], ps_e0[:, 0:npx_e], AF.Square)
                    nc.scalar.activation(sq1t[:, 0:npx_e], ps_e1[:, 0:npx_e], AF.Square)
                    ps_stm = p3ps.tile([1, NPX], F32, tag="small")
                    nc.tensor.matmul(ps_stm[0:1, 0:npx_e], stat256[:, 0:1],
                                     t_strip[:, 0, bslot, 0:npx_e], start=True, stop=False)
                    nc.tensor.matmul(ps_stm[0:1, 0:npx_e], stat256[:, 0:1],
                                     t_strip[:, 1, bslot, 0:npx_e], start=False, stop=True)
                    mu_t = p3.tile([1, NPX], F32R, tag="mu_t", bufs=5)
                    nc.vector.tensor_copy(mu_t[:, 0:npx_e], ps_stm[0:1, 0:npx_e])
                    ps_sts = p3ps.tile([1, NPX], F32, tag="small")
                    nc.tensor.matmul(ps_sts[0:1, 0:npx_e], stat256[:, 1:2],
                                     sq0[:, 0:npx_e], start=True, stop=False)
                    nc.tensor.matmul(ps_sts[0:1, 0:npx_e], stat256[:, 1:2],
                                     sq1t[:, 0:npx_e], start=False, stop=True)
                    musq = p3.tile([1, NPX], F32, tag="musq", bufs=1)
                    nc.gpsimd.tensor_tensor(musq[:, 0:npx_e], mu_t[:, 0:npx_e],
                                            mu_t[:, 0:npx_e], op=ALU.mult)
                    varr3 = p3.tile([1, NPX], F32, tag="varr3", bufs=2)
                    nc.vector.scalar_tensor_tensor(varr3[:, 0:npx_e],
                                                   ps_sts[0:1, 0:npx_e], EPS,
                                                   musq[:, 0:npx_e],
                                                   op0=ALU.add, op1=ALU.subtract)
                    rcp3 = p3.tile([1, NPX], F32, tag="rcp3", bufs=2)
                    nc.vector.reciprocal_approx_fast(out=rcp3[:, 0:npx_e],
                                                     in_=varr3[:, 0:npx_e])
                    r_t = p3.tile([1, NPX], F32R, tag="r_t", bufs=5)
                    nc.scalar.activation(r_t[:, 0:npx_e], rcp3[:, 0:npx_e], AF.Sqrt)
                    return mu_t, r_t

                def stage_a_post(bslot, re, nrows, slo, mu_t, r_t):
                    npx_e = nrows * W
                    ps_bmu = p3ps.tile([128, NPX], F32, tag="bc", bufs=2)
                    nc.tensor.matmul(ps_bmu[:, 0:npx_e], ones128,
                                     mu_t[:, 0:npx_e], start=True, stop=True)
                    ps_br = p3ps.tile([128, NPX], F32, tag="bc", bufs=2)
                    nc.tensor.matmul(ps_br[:, 0:npx_e], ones128,
                                     r_t[:, 0:npx_e], start=True, stop=True)
                    vns = []
                    for h in range(2):
                        dh = p3.tile([128, NPX], F32, tag="dh")
                        nc.vector.tensor_tensor(dh[:, 0:npx_e], t_strip[:, h, bslot, 0:npx_e],
                                                ps_bmu[:, 0:npx_e], op=ALU.subtract)
                        vn = p3.tile([128, NPX], F32R, tag=f"vn{h}")
                        nc.vector.tensor_tensor(vn[:, 0:npx_e], dh[:, 0:npx_e],
                                                ps_br[:, 0:npx_e], op=ALU.mult)
                        vns.append(vn)
                    for mt in range(2):
                        ps_f = p3ps.tile([128, NPX], F32, tag="work", bufs=2)
                        nc.tensor.matmul(ps_f[:, 0:npx_e], wf1v[:, 0, mt],
                                         vns[0][:, 0:npx_e], start=True, stop=False)
                        nc.tensor.matmul(ps_f[:, 0:npx_e], wf1v[:, 1, mt],
                                         vns[1][:, 0:npx_e], start=False, stop=True)
                        fxs = p3.tile([128, NPX], BF16, tag="fxs")
                        nc.sync.dma_start(out=fxs[:, 0:npx_e],
                                          in_=fx_dram[mt, :, re * W:re * W + npx_e])
                        dstf = f1win[:, mt, slo + 1:slo + 1 + nrows, 1:1 + W]
                        nc.vector.tensor_tensor(
                            dstf, ps_f[:, 0:npx_e].rearrange("p (a b) -> p a b", a=nrows),
                            fxs[:, 0:npx_e].rearrange("p (a b) -> p a b", a=nrows),
                            op=ALU.add)

                def stage_b(rg, nrg, slg):
                    npx_g = nrg * W
                    gsb = p3.tile([128, 2, NPX], BF16, tag="gsb")
                    for ct in range(2):
                        ps_g = p3ps.tile([128, NPX], F32, tag="work", bufs=2)
                        for t_i, (dy, dx) in enumerate(TAPS):
                            rhs = f1win[:, ct, slg + dy:slg + dy + nrg, 1 + dx:1 + dx + W]
                            nc.tensor.matmul(ps_g[:, 0:npx_g], wdw[:, ct, t_i], rhs,
                                             start=(t_i == 0), stop=(t_i == 8))
                        nc.scalar.activation(gsb[:, ct, 0:npx_g], ps_g[:, 0:npx_g], AF.Gelu,
                                             bias=bias_g[:, ct])
                    for mt in range(2):
                        ps_out = p3ps.tile([128, NPX], F32, tag="work", bufs=2)
                        nc.tensor.matmul(ps_out[:, 0:npx_g], wf2[:, 0, mt], gsb[:, 0, 0:npx_g],
                                         start=True, stop=False)
                        nc.tensor.matmul(ps_out[:, 0:npx_g], wf2[:, 1, mt], gsb[:, 1, 0:npx_g],
                                         start=False, stop=True)
                        osb = p3.tile([128, NPX], F32, tag="osb")
                        nc.scalar.copy(osb[:, 0:npx_g], ps_out[:, 0:npx_g])
                        nc.sync.dma_start(out=out_dram[mt, :, rg * W:rg * W + npx_g],
                                          in_=osb[:, 0:npx_g])

                for s_i in range(NSTRIP):
                    r0 = 16 * s_i
                    last = (s_i == NSTRIP - 1)
                    if s_i > 0:
                        nc.gpsimd.tensor_copy(owin2[:, 0:2], owin2[:, 16:18])
                        nc.gpsimd.tensor_copy(f1win[:, :, 0:3], f1win[:, :, 16:19])
                    blocks = []
                    for b_i in range(NBLK):
                        rb = r0 + BLK_ROWS * b_i
                        sl_w = 4 * b_i + 2
                        src = vo_store[S:128, rb * W:(rb + 4) * W]
                        nc.vector.tensor_copy(owin2[0:S, sl_w:sl_w + 4, 1:1 + W],
                                              src.rearrange("p (a b) -> p a b", a=4))
                        nc.gpsimd.tensor_copy(owin2[S:128, sl_w - 1:sl_w + 3, 1:1 + W],
                                              src.rearrange("p (a b) -> p a b", a=4))
                        if s_i == 0 and b_i == 0:
                            blk = (b_i, 0, 3, 2)
                        else:
                            blk = (b_i, rb - 1, 4, 4 * b_i + 1)
                        mu_t, r_t = stage_a_pre(*blk)
                        blocks.append((blk, mu_t, r_t))
                    if last:
                        blk = (4, 127, 1, 17)
                        mu_t, r_t = stage_a_pre(*blk)
                        blocks.append((blk, mu_t, r_t))
                    for blk, mu_t, r_t in blocks:
                        stage_a_post(*blk, mu_t, r_t)
                    for b_i in range(NBLK):
                        rb = r0 + BLK_ROWS * b_i
                        if s_i == 0 and b_i == 0:
                            stage_b(0, 2, 3)
                        else:
                            stage_b(rb - 2, 4, 4 * b_i + 1)
                stage_b(126, 2, 17)
            p3w_stack.close()
    return nc


def _prep_host(inputs):
    f32 = np.float32
    w_cq = np.asarray(inputs["w_cq"], f32)
    w_ckv = np.asarray(inputs["w_ckv"], f32)
    ln_q_w = np.asarray(inputs["ln_q_w"], f32); ln_q_b = np.asarray(inputs["ln_q_b"], f32)
    ln_kv_w = np.asarray(inputs["ln_kv_w"], f32); ln_kv_b = np.asarray(inputs["ln_kv_b"], f32)
    w_kv = np.asarray(inputs["w_kv"], f32)
    w_kvdw = np.asarray(inputs["w_kvdw"], f32)
    w_q = np.asarray(inputs["w_q"], f32)
    temperature = np.asarray(inputs["temperature"], f32)
    w_po = np.asarray(inputs["w_po"], f32)
    w_expand = np.asarray(inputs["w_expand"], f32)
    ln_out_w = np.asarray(inputs["ln_out_w"], f32); ln_out_b = np.asarray(inputs["ln_out_b"], f32)
    w_ffn1 = np.asarray(inputs["w_ffn1"], f32)
    w_ffn_dw = np.asarray(inputs["w_ffn_dw"], f32)
    w_ffn2 = np.asarray(inputs["w_ffn2"], f32)

    d = {}

    def conv_lhsT(wc, scale=1.0):
        a = np.zeros((128, 2, 9, S + 1), f32)
        for kt in range(2):
            blk = wc[:, kt * 128:(kt + 1) * 128]           # [S, 128, 3, 3]
            a[:, kt, :, :S] = blk.transpose(1, 2, 3, 0).reshape(128, 9, S)
            a[:, kt, :, S] = blk.sum(axis=0).reshape(128, 9)
        return a * scale
    wcq8 = np.zeros((128, 9, 2, 128), f32)
    wcq8[:, :, :, :S + 1] = conv_lhsT(w_cq, SC8).transpose(0, 2, 1, 3)
    d["wcq8"] = wcq8.astype(F8NP)
    d["wckv"] = conv_lhsT(w_ckv)

    # q conv (fp8, scaled): K=128 DoubleRow layout over (low/high rows x 2 k-tiles)
    # low partitions see row(slot)+i, high partitions see row(slot)+1+i;
    # k-tile1 reads one slot later. dy=-1 -> (k0,low), dy=0 -> (k0,high),
    # dy=+1 -> (k1,high); (k1,low) gets zero weights.
    wq = (w_q * ln_q_w[None, :, None, None]) * SC8        # [S out, S in, 3, 3]
    wq2 = np.zeros((128, 3, 2, S), f32)
    for dxi in range(3):
        wq2[0:S, dxi, 0, :] = wq[:, :, 0, dxi].T
        wq2[S:128, dxi, 0, :] = wq[:, :, 1, dxi].T
        wq2[S:128, dxi, 1, :] = wq[:, :, 2, dxi].T
    d["wq2"] = wq2.astype(F8NP)
    d["bias_q"] = ((w_q * ln_q_b[None, :, None, None]).sum(axis=(1, 2, 3)) * SC8).reshape(S, 1)

    # kv conv (f32r, dy-stacked)
    w_kv_g = w_kv[:, :, 0, 0] * ln_kv_w[None, :]
    w_kv_eff = w_kvdw[:, 0][:, None] * w_kv_g[:, :, None, None]   # [2S, S, 3, 3]
    d["bias_kv"] = (w_kvdw[:, 0].sum(axis=(1, 2)) * (w_kv[:, :, 0, 0] @ ln_kv_b)).reshape(2 * S, 1)
    wkva = np.zeros((128, 3, 128), f32)
    wkvb = np.zeros((S, 3, 128), f32)
    for dxi in range(3):
        wkva[0:S, dxi, :] = w_kv_eff[:, :, 0, dxi].T
        wkva[S:128, dxi, :] = w_kv_eff[:, :, 1, dxi].T
        wkvb[:, dxi, :] = w_kv_eff[:, :, 2, dxi].T
    d["wkva"] = wkva
    d["wkvb"] = wkvb

    # expand conv folded with project_out (bf16, dy-stacked)
    wpo_m = w_po[:, :, 0, 0]                              # [S out(i), S in(j)]
    w_eff = np.einsum("cit,ij->cjt", w_expand.reshape(C, S, 9), wpo_m).reshape(C, S, 3, 3)
    wexpa = np.zeros((128, 3, C), f32)
    wexpb = np.zeros((S, 3, C), f32)
    for dxi in range(3):
        wexpa[0:S, dxi, :] = w_eff[:, :, 0, dxi].T
        wexpa[S:128, dxi, :] = w_eff[:, :, 1, dxi].T
        wexpb[:, dxi, :] = w_eff[:, :, 2, dxi].T
    d["wexpa"] = wexpa.astype(ml_dtypes.bfloat16)
    d["wexpb"] = wexpb.astype(ml_dtypes.bfloat16)

    w1 = w_ffn1[:, :, 0, 0]
    w1x = w1[:, :C]
    w1v = w1[:, C:] * ln_out_w[None, :]

    def one_by_one_lhsT(wm):
        a = np.zeros((128, 2, 2, 128), f32)
        for kt in range(2):
            for mt in range(2):
                a[:, kt, mt, :] = wm[mt * 128:(mt + 1) * 128, kt * 128:(kt + 1) * 128].T
        return a
    d["wf1x"] = one_by_one_lhsT(w1x)
    d["wf1v"] = one_by_one_lhsT(w1v)
    bias_f1 = w1[:, C:] @ ln_out_b
    dw_t = w_ffn_dw[:, 0].reshape(C, 9)
    d["bias_g"] = np.ascontiguousarray(
        (bias_f1 * dw_t.sum(1)).reshape(2, 128, 1).transpose(1, 0, 2))
    wdw = np.zeros((128, 2, 9, 128), f32)
    for ct in range(2):
        for t in range(9):
            np.fill_diagonal(wdw[:, ct, t, :], dw_t[ct * 128:(ct + 1) * 128, t])
    d["wdw"] = wdw.astype(ml_dtypes.bfloat16)
    d["wf2"] = one_by_one_lhsT(w_ffn2[:, :, 0, 0]).astype(ml_dtypes.bfloat16)
    stat_cq = np.zeros((S + 1, 1), f32)
    stat_cq[:S, 0] = 1.0 / S
    stat_cq[S, 0] = -1.0 / (S * S)
    d["stat_cq"] = stat_cq
    # brs row0 = rsqrt (Act-written), row32 = raw channel sum (DVE-written)
    bc2 = np.zeros((33, 128), f32)
    bc2[0, S:128] = 1.0
    bc2[32, 0:S] = 1.0 / S
    d["bc2"] = bc2
    d["ones128"] = np.ones((1, 128), f32)
    stat256 = np.zeros((128, 2), f32)
    stat256[:, 0] = 1.0 / C
    stat256[:, 1] = 1.0 / C
    d["stat256"] = stat256
    d["tempv"] = np.repeat(temperature.reshape(HEADS), S // HEADS).reshape(S, 1).astype(f32)
    mask = np.zeros((S, S), f32)
    for h in range(HEADS):
        mask[h * 8:(h + 1) * 8, h * 8:(h + 1) * 8] = 1.0
    d["maskbd"] = mask
    d["identr"] = np.eye(S, dtype=f32)
    d["zeros8"] = np.zeros((128, 2600), f32).astype(F8NP)
    d["zeros"] = np.zeros((128, 2600), f32)
    d["zerosb"] = np.zeros((128, 2600), f32).astype(ml_dtypes.bfloat16)
    d["onesr"] = np.ones((1, S), f32)
    return d


def _pad_input(x):
    """[C,H,W] f32 -> [128, 2, H+4, WP] zero-padded, partition-major"""
    out = np.zeros((128, 2, H + 4, WP), np.float32)
    out[:, :, 2:H + 2, 1:W + 1] = x.reshape(2, 128, H, W).transpose(1, 0, 2, 3)
    return out


def make_in_maps(inputs):
    d = _prep_host(inputs)
    x = np.asarray(inputs["x"], np.float32)
    y = np.asarray(inputs["y"], np.float32)
    in_maps = []
    for i in range(B):
        m = dict(d)
        xp = _pad_input(x[i])
        m["x"] = xp
        m["x8"] = xp.astype(F8NP)
        m["y"] = _pad_input(y[i])
        in_maps.append(m)
    return in_maps


def kernel(**inputs):
    key = "nc"
    if key not in _CACHED:
        nc = build_nc()
        nc.finalize()
        _CACHED[key] = nc
    nc = _CACHED[key]
    in_maps = make_in_maps(inputs)
    res = run_bass_kernel_spmd(nc, in_maps, list(range(B)))
    out = np.stack([res.results[i]["out"].reshape(C, H, W) for i in range(B)])
    return out.astype(np.float32)


# revision 52
# speedup vs baseline: 1.0457x; 1.0273x over previous
# Trainium2 Bass kernel for ChannelAttentionBlock (B=8,C=256,H=W=128,S=64,HEADS=8)
# Data-parallel over batch: 1 sample per NeuronCore, 8 cores.
#
# v2: fp8 DoubleRow for the q-side convs (scale-invariant attention path),
# dy-stacked kv/expand convs, wpo folded into expand, strip-batched LN sqrt
# (keeps the Act table stable), bf16 FFN tail, engine rebalance.
import contextlib
import numpy as np
import ml_dtypes

import concourse.bass as bass
from concourse import bacc
import concourse.mybir as mybir
from concourse.bass_utils import run_bass_kernel_spmd
from concourse.tile import TileContext

F32R = mybir.dt.float32r
F32 = mybir.dt.float32
BF16 = mybir.dt.bfloat16
FP8 = mybir.dt.float8e4
F8NP = mybir.dt.np(mybir.dt.float8e4)
AF = mybir.ActivationFunctionType
ALU = mybir.AluOpType
DRMODE = mybir.MatmulPerfMode.DoubleRow

B, C, H, W = 8, 256, 128, 128
S = 64
HEADS = 8
HW = H * W
WP = W + 2          # padded row width
NSTRIP = H // 16
BLK_ROWS = 4
NBLK = 4
NPX = BLK_ROWS * W  # 512
EPS = 1e-5
SC8 = 64.0          # fp8 weight scale (washes out in LN / q-normalization)
EPS_CQ = EPS * SC8 * SC8

TAPS = [(dy, dx) for dy in (-1, 0, 1) for dx in (-1, 0, 1)]

_CACHED = {}


def _ins_dim(ap, entry):
    """Insert a [stride, count] dim right after the partition dim of an AP."""
    dims = [list(d) for d in ap.ap]
    return bass.AP(tensor=ap.tensor, offset=ap.offset,
                   ap=[dims[0], list(entry)] + dims[1:])


def build_nc():
    nc = bacc.Bacc("TRN2", target_bir_lowering=False, debug=False)

    # ------------- DRAM parameters (host layouts) -------------
    x_in = nc.dram_tensor("x", [128, 2, H + 4, WP], F32R, kind="ExternalInput")
    x8_in = nc.dram_tensor("x8", [128, 2, H + 4, WP], FP8, kind="ExternalInput")
    y_in = nc.dram_tensor("y", [128, 2, H + 4, WP], F32R, kind="ExternalInput")
    wcq8_in = nc.dram_tensor("wcq8", [128, 9, 2, 128], FP8, kind="ExternalInput")
    wckv_in = nc.dram_tensor("wckv", [128, 2, 9, S + 1], F32R, kind="ExternalInput")
    wq2_in = nc.dram_tensor("wq2", [128, 3, 2, S], FP8, kind="ExternalInput")
    wkva_in = nc.dram_tensor("wkva", [128, 3, 128], F32R, kind="ExternalInput")
    wkvb_in = nc.dram_tensor("wkvb", [S, 3, 128], F32R, kind="ExternalInput")
    wexpa_in = nc.dram_tensor("wexpa", [128, 3, C], BF16, kind="ExternalInput")
    wexpb_in = nc.dram_tensor("wexpb", [S, 3, C], BF16, kind="ExternalInput")
    wf1x_in = nc.dram_tensor("wf1x", [128, 2, 2, 128], F32R, kind="ExternalInput")
    wf1v_in = nc.dram_tensor("wf1v", [128, 2, 2, 128], BF16, kind="ExternalInput")
    wdw_in = nc.dram_tensor("wdw", [128, 2, 9, 128], BF16, kind="ExternalInput")
    wf2_in = nc.dram_tensor("wf2", [128, 2, 2, 128], BF16, kind="ExternalInput")
    stat_cq_in = nc.dram_tensor("stat_cq", [S + 1, 1], F32R, kind="ExternalInput")
    bc2_in = nc.dram_tensor("bc2", [33, 128], F32R, kind="ExternalInput")
    ones128_in = nc.dram_tensor("ones128", [1, 128], BF16, kind="ExternalInput")
    stat256_in = nc.dram_tensor("stat256", [128, 2], F32R, kind="ExternalInput")
    bias_q_in = nc.dram_tensor("bias_q", [S, 1], F32, kind="ExternalInput")
    bias_kv_in = nc.dram_tensor("bias_kv", [128, 1], F32, kind="ExternalInput")
    bias_g_in = nc.dram_tensor("bias_g", [128, 2, 1], F32, kind="ExternalInput")
    temp_in = nc.dram_tensor("tempv", [S, 1], F32, kind="ExternalInput")
    mask_in = nc.dram_tensor("maskbd", [S, S], F32R, kind="ExternalInput")
    identr_in = nc.dram_tensor("identr", [S, S], F32R, kind="ExternalInput")
    identm_in = nc.dram_tensor("identm", [128, 128], BF16, kind="ExternalInput")
    ones_in = nc.dram_tensor("onesr", [1, S], F32R, kind="ExternalInput")
    zeros8_in = nc.dram_tensor("zeros8", [128, 2600], FP8, kind="ExternalInput")
    zeros_in = nc.dram_tensor("zeros", [128, 2600], F32R, kind="ExternalInput")
    zerosb_in = nc.dram_tensor("zerosb", [128, 2600], BF16, kind="ExternalInput")

    fx_dram = nc.dram_tensor("fx_dram", [2, 128, HW], BF16)
    out_dram = nc.dram_tensor("out", [2, 128, HW], F32, kind="ExternalOutput")

    with TileContext(nc) as tc:
        with tc.tile_pool(name="persist", bufs=1) as persist:
            qk_store = persist.tile([128, HW], BF16, tag="qk_store")
            qkT = persist.tile([128, H, 128], BF16, tag="qkT")
            vo_store = persist.tile([128, HW], BF16, tag="vo_store")
            rq2 = persist.tile([S, 1], F32, tag="rq2")
            rk2 = persist.tile([S, 1], F32, tag="rk2")
            nc.vector.memset(rq2, 0.0)
            nc.vector.memset(rk2, 0.0)

            # ================= PHASE 1 =================
            with tc.tile_pool(name="p1w", bufs=1) as p1w, \
                 tc.tile_pool(name="p1", bufs=2) as p1, \
                 tc.tile_pool(name="p1ps", bufs=1, space="PSUM") as p1ps:
                wcq8 = p1w.tile([128, 9, 2, 128], FP8, tag="wcq8")
                nc.sync.dma_start(out=wcq8, in_=wcq8_in[:, :, :, :])
                wckv = p1w.tile([128, 2, 9, S + 1], F32R, tag="wckv")
                nc.sync.dma_start(out=wckv, in_=wckv_in[:, :, :, :])
                wq2 = p1w.tile([128, 3, 2, S], FP8, tag="wq2")
                nc.sync.dma_start(out=wq2, in_=wq2_in[:, :, :, :])
                wkva = p1w.tile([128, 3, 128], F32R, tag="wkva")
                nc.sync.dma_start(out=wkva, in_=wkva_in[:, :, :])
                wkvb = p1w.tile([S, 3, 128], F32R, tag="wkvb")
                nc.sync.dma_start(out=wkvb, in_=wkvb_in[:, :, :])
                wf1x = p1w.tile([128, 2, 2, 128], F32R, tag="wf1x")
                nc.sync.dma_start(out=wf1x, in_=wf1x_in[:, :, :, :])
                stat_cq = p1w.tile([S + 1, 1], F32R, tag="stat_cq")
                nc.sync.dma_start(out=stat_cq, in_=stat_cq_in[:, :])
                bc2 = p1w.tile([33, 128], F32R, tag="bc2")
                nc.sync.dma_start(out=bc2, in_=bc2_in[:, :])
                # broadcast rhs: row 0 = rsqrt (Act), row 32 = channel sum (DVE);
                # rows 1..31 stay zero (quad-aligned partition starts only)
                brs = p1w.tile([33, NPX], F32R, tag="brs")
                nc.sync.dma_start(out=brs, in_=zeros_in[0:33, 0:NPX])
                bias_q = p1w.tile([S, 1], F32, tag="bias_q")
                nc.sync.dma_start(out=bias_q, in_=bias_q_in[:, :])
                bias_kv = p1w.tile([128, 1], F32, tag="bias_kv")
                nc.sync.dma_start(out=bias_kv, in_=bias_kv_in[:, :])

                xwin = p1w.tile([128, 2, 18, WP], F32R, tag="xwin")
                xwin8 = p1w.tile([128, 2, 18, WP], FP8, tag="xwin8")
                ywin = p1w.tile([128, 2, 18, WP], F32R, tag="ywin")
                # nq2: LN'd xq (fp8); partitions 0:64 = row(slot), 64:128 = row(slot)+1
                nq2 = p1w.tile([128, 19, WP], FP8, tag="nq2")
                nc.sync.dma_start(out=nq2.rearrange("p a b -> p (a b)"),
                                  in_=zeros8_in[:, :19 * WP])
                # nkv: LN'd ykv; partitions 0:64 = row(slot), 64:128 = row(slot)+1
                nkv = p1w.tile([128, 19, WP], F32R, tag="nkv")
                nc.sync.dma_start(out=nkv.rearrange("p a b -> p (a b)"),
                                  in_=zeros_in[:, :19 * WP])

                def ln_tail(ps_c, eps, t_on_act, write_out):
                    """LN stats+apply for conv PSUM ps_c [S+1, NPX]."""
                    t_sb = p1.tile([S, NPX], F32R, tag="t_sb")
                    if t_on_act:
                        nc.scalar.copy(t_sb, ps_c[0:S])
                    else:
                        nc.vector.tensor_copy(t_sb, ps_c[0:S])
                    sq_sb = p1.tile([S + 1, NPX], F32R, tag="sq_sb")
                    nc.scalar.activation(sq_sb, ps_c[0:S + 1], AF.Square)
                    ps_v = p1ps.tile([128, NPX], F32, tag="bc", bufs=2)
                    nc.tensor.matmul(ps_v[0:1], stat_cq, sq_sb, start=True, stop=True)
                    varr = p1.tile([1, NPX], F32, tag="varr", bufs=1)
                    nc.vector.tensor_scalar_add(varr, ps_v[0:1], eps)
                    rcpv = p1.tile([1, NPX], F32, tag="rcpv", bufs=1)
                    nc.vector.reciprocal_approx_fast(out=rcpv, in_=varr)
                    nc.scalar.activation(brs[0:1], rcpv, AF.Sqrt)
                    nc.vector.tensor_copy(brs[32:33], ps_c[S:S + 1])
                    ps_b = p1ps.tile([128, NPX], F32, tag="bc", bufs=2)
                    nc.tensor.matmul(ps_b, bc2, brs, start=True, stop=True)
                    d_sb = p1.tile([S, NPX], F32, tag="d_sb")
                    nc.vector.tensor_tensor(d_sb, t_sb, ps_b[0:S], op=ALU.subtract)
                    write_out(d_sb, ps_b)

                def conv_part(s_i, b_i):
                    rb = 16 * s_i + BLK_ROWS * b_i
                    if s_i == 0 and b_i == 0:
                        nc.sync.dma_start(out=xwin[:, :, 0:6], in_=x_in[:, :, 1:7])
                        nc.sync.dma_start(out=xwin8[:, :, 0:6], in_=x8_in[:, :, 1:7])
                        nc.sync.dma_start(out=ywin[:, :, 0:6], in_=y_in[:, :, 1:7])
                    else:
                        sl = 4 * b_i + 2
                        nc.sync.dma_start(out=xwin[:, :, sl:sl + 4], in_=x_in[:, :, rb + 3:rb + 7])
                        nc.sync.dma_start(out=xwin8[:, :, sl:sl + 4], in_=x8_in[:, :, rb + 3:rb + 7])
                        nc.sync.dma_start(out=ywin[:, :, sl:sl + 4], in_=y_in[:, :, rb + 3:rb + 7])
                    if b_i == 3 and s_i < NSTRIP - 1:
                        # early carry for next strip (sources are DMA slots 16:18)
                        nc.vector.tensor_copy(xwin[:, :, 0:2], xwin[:, :, 16:18])
                        nc.gpsimd.tensor_copy(xwin8[:, :, 0:2], xwin8[:, :, 16:18])
                        nc.vector.tensor_copy(ywin[:, :, 0:2], ywin[:, :, 16:18])
                    # cq conv (fp8 DoubleRow, 9 taps)
                    ps_cq = p1ps.tile([128, NPX], F32, tag="ps_cq", bufs=2)
                    for t_i, (dy, dx) in enumerate(TAPS):
                        sl0 = 4 * b_i + 1 + dy
                        rhs = xwin8[:, :, sl0:sl0 + 4, 1 + dx:1 + dx + W]
                        nc.tensor.matmul(ps_cq, wcq8[:, t_i], rhs,
                                         start=(t_i == 0), stop=(t_i == 8),
                                         perf_mode=DRMODE)
                    # ckv conv (f32r, 18 matmuls)
                    ps_ckv = p1ps.tile([S + 1, NPX], F32, tag="ps_ckv", bufs=2)
                    first = True
                    for kt in range(2):
                        for t_i, (dy, dx) in enumerate(TAPS):
                            sl0 = 4 * b_i + 1 + dy
                            rhs = ywin[:, kt, sl0:sl0 + 4, 1 + dx:1 + dx + W]
                            nc.tensor.matmul(ps_ckv, wckv[:, kt, t_i], rhs,
                                             start=first, stop=(kt == 1 and t_i == 8))
                            first = False
                    return ps_cq, ps_ckv

                def ln_part(s_i, b_i, ps_cq, ps_ckv):
                    rb = 16 * s_i + BLK_ROWS * b_i
                    if b_i == 0 and s_i > 0:
                        nc.gpsimd.tensor_copy(nq2[:, 0:2], nq2[:, 16:18])
                        nc.vector.tensor_copy(nkv[:, 0:2], nkv[:, 16:18])
                    sl_w = 4 * b_i + 2   # write slot of row rb

                    def write_xq(d_sb, ps_b, sl_w=sl_w):
                        dst = nq2[0:S, sl_w:sl_w + 4, 1:1 + W]
                        nc.vector.tensor_tensor(
                            dst, d_sb.rearrange("p (a b) -> p a b", a=4),
                            ps_b[64:128].rearrange("p (a b) -> p a b", a=4),
                            op=ALU.mult)
                        nc.gpsimd.tensor_copy(nq2[S:128, sl_w - 1:sl_w + 3, 1:1 + W],
                                              nq2[0:S, sl_w:sl_w + 4, 1:1 + W])

                    def write_kv(d_sb, ps_b, sl_w=sl_w):
                        dst = nkv[0:S, sl_w:sl_w + 4, 1:1 + W]
                        nc.vector.tensor_tensor(
                            dst, d_sb.rearrange("p (a b) -> p a b", a=4),
                            ps_b[64:128].rearrange("p (a b) -> p a b", a=4),
                            op=ALU.mult)
                        nc.gpsimd.tensor_copy(nkv[S:128, sl_w - 1:sl_w + 3, 1:1 + W],
                                              nkv[0:S, sl_w:sl_w + 4, 1:1 + W])

                    ln_tail(ps_cq, EPS_CQ, True, write_xq)
                    ln_tail(ps_ckv, EPS, False, write_kv)

                def fx_part(s_i, b_i):
                    rb = 16 * s_i + BLK_ROWS * b_i
                    for mt in range(2):
                        ps_fx = p1ps.tile([128, NPX], F32, tag="mix", bufs=2)
                        for kt in range(2):
                            rhs = xwin[:, kt, 4 * b_i + 1:4 * b_i + 5, 1:1 + W]
                            nc.tensor.matmul(ps_fx, wf1x[:, kt, mt],
                                             rhs, start=(kt == 0), stop=(kt == 1))
                        fx_sb = p1.tile([128, NPX], BF16, tag="fx_sb")
                        nc.scalar.copy(fx_sb, ps_fx)
                        nc.sync.dma_start(out=fx_dram[mt, :, rb * W:(rb + 4) * W], in_=fx_sb)

                def qkv_part(rq, nrows, sl_base):
                    npx_q = nrows * W
                    s_m1 = sl_base - 1
                    s_p1 = sl_base + 1
                    # q: fp8; 3 K=128 DR matmuls cover all 9 taps
                    ps_q = p1ps.tile([128, NPX], F32, tag="mix", bufs=2)
                    for dxi in range(3):
                        base = nq2[:, s_m1:s_m1 + nrows, dxi:dxi + W]
                        rhs = _ins_dim(base, [WP, 2])
                        nc.tensor.matmul(ps_q[0:S, 0:npx_q], wq2[:, dxi], rhs,
                                         start=(dxi == 0), stop=(dxi == 2),
                                         perf_mode=DRMODE)
                    # kv: f32r; 3 dy-pair streams (K=128) + 3 singles (K=64)
                    ps_kv = p1ps.tile([128, NPX], F32, tag="mix", bufs=2)
                    for dxi in range(3):
                        rhs = nkv[:, s_m1:s_m1 + nrows, dxi:dxi + W]
                        nc.tensor.matmul(ps_kv[:, 0:npx_q], wkva[:, dxi], rhs,
                                         start=(dxi == 0), stop=False)
                    for dxi in range(3):
                        rhs = nkv[0:S, s_p1:s_p1 + nrows, dxi:dxi + W]
                        nc.tensor.matmul(ps_kv[:, 0:npx_q], wkvb[:, dxi], rhs,
                                         start=False, stop=(dxi == 2))
                    c0 = rq * W
                    nc.scalar.activation(qk_store[0:S, c0:c0 + npx_q], ps_q[0:S, 0:npx_q],
                                         AF.Identity, bias=bias_q)
                    nc.scalar.activation(qk_store[S:128, c0:c0 + npx_q], ps_kv[0:S, 0:npx_q],
                                         AF.Identity, bias=bias_kv[0:S])
                    nc.scalar.activation(vo_store[0:S, c0:c0 + npx_q], ps_kv[S:128, 0:npx_q],
                                         AF.Identity, bias=bias_kv[S:128])
                    nc.scalar.dma_start_transpose(out=qkT[:, rq:rq + nrows, :],
                                                  in_=qk_store[:, c0:c0 + npx_q])
                    scr = p1.tile([S, NPX], BF16, tag="scr")
                    q_acc = p1.tile([S, 1], F32, tag="q_acc")
                    nc.scalar.activation(scr[:, 0:npx_q], qk_store[0:S, c0:c0 + npx_q],
                                         AF.Square, accum_out=q_acc)
                    nc.vector.tensor_tensor(rq2, rq2, q_acc, op=ALU.add)
                    scr2 = p1.tile([S, NPX], BF16, tag="scr2")
                    k_acc = p1.tile([S, 1], F32, tag="k_acc")
                    nc.scalar.activation(scr2[:, 0:npx_q], qk_store[S:128, c0:c0 + npx_q],
                                         AF.Square, accum_out=k_acc)
                    nc.vector.tensor_tensor(rk2, rk2, k_acc, op=ALU.add)

                # software-pipelined block loop: convs of block b overlap the
                # LN tail of block b-1 and the q/kv convs of block b-2, giving
                # the serial LN chain ~2 block periods of slack
                def qkv_args(ps_i, pb_i):
                    rb = 16 * ps_i + BLK_ROWS * pb_i
                    if ps_i == 0 and pb_i == 0:
                        return (0, 3, 2)
                    return (rb - 1, 4, 4 * pb_i + 1)

                pend = []
                for s_i in range(NSTRIP):
                    for b_i in range(NBLK):
                        cur = conv_part(s_i, b_i)
                        if pend:
                            ps_i, pb_i, pcq, pckv = pend[-1]
                            ln_part(ps_i, pb_i, pcq, pckv)
                        fx_part(s_i, b_i)
                        if len(pend) >= 1:
                            ps_i, pb_i, _, _ = pend.pop(0)
                            qkv_part(*qkv_args(ps_i, pb_i))
                        pend.append((s_i, b_i) + cur)
                ps_i, pb_i, pcq, pckv = pend[-1]
                ln_part(ps_i, pb_i, pcq, pckv)
                for ps_i, pb_i, _, _ in pend:
                    qkv_part(*qkv_args(ps_i, pb_i))
                # epilogue: q/kv row 127 (slot of row 127 = 17; slot 18 zero)
                qkv_part(127, 1, 17)

            # -------- phase-3 weights prefetch (overlaps phase 2) --------
            p3w_stack = contextlib.ExitStack()
            p3w = p3w_stack.enter_context(tc.tile_pool(name="p3w", bufs=1))
            wexpa = p3w.tile([128, 3, C], BF16, tag="wexpa")
            nc.sync.dma_start(out=wexpa, in_=wexpa_in[:, :, :])
            wexpb = p3w.tile([S, 3, C], BF16, tag="wexpb")
            nc.sync.dma_start(out=wexpb, in_=wexpb_in[:, :, :])
            wf1v = p3w.tile([128, 2, 2, 128], BF16, tag="wf1v")
            nc.sync.dma_start(out=wf1v, in_=wf1v_in[:, :, :, :])
            wdw = p3w.tile([128, 2, 9, 128], BF16, tag="wdw")
            nc.sync.dma_start(out=wdw, in_=wdw_in[:, :, :, :])
            wf2 = p3w.tile([128, 2, 2, 128], BF16, tag="wf2")
            nc.sync.dma_start(out=wf2, in_=wf2_in[:, :, :, :])
            stat256 = p3w.tile([128, 2], F32R, tag="stat256")
            nc.sync.dma_start(out=stat256, in_=stat256_in[:, :])
            ones128 = p3w.tile([1, 128], BF16, tag="ones128")
            nc.sync.dma_start(out=ones128, in_=ones128_in[:, :])
            bias_g = p3w.tile([128, 2, 1], F32, tag="bias_g")
            nc.sync.dma_start(out=bias_g, in_=bias_g_in[:, :, :])
            # owin2: o rows; partitions 0:64 = row(slot), 64:128 = row(slot)+1
            owin2 = p3w.tile([128, 19, WP], BF16, tag="owin2")
            nc.sync.dma_start(out=owin2.rearrange("p a b -> p (a b)"),
                              in_=zerosb_in[:, :19 * WP])
            # f1win: slot i = row (r0-3)+i ; slot 19 always zero
            f1win = p3w.tile([128, 2, 20, WP], BF16, tag="f1win")
            for _h in range(2):
                nc.sync.dma_start(out=f1win[:, _h].rearrange("p a b -> p (a b)"),
                                  in_=zerosb_in[:, :20 * WP])
            t_strip = p3w.tile([128, 2, 5, NPX], F32R, tag="t_strip")

            # ================= PHASE 2: attention =================
            with tc.tile_pool(name="p2", bufs=2) as p2, \
                 tc.tile_pool(name="p2one", bufs=1) as p2one, \
                 tc.tile_pool(name="p2ps", bufs=2, space="PSUM") as p2ps:
                NACC = 4
                gs = []
                for j in range(NACC):
                    g_acc = p2ps.tile([S, S], F32, tag=f"g{j}", bufs=1, name=f"g_acc{j}")
                    gs.append(g_acc)
                for tb in range(H):
                    j = tb % NACC
                    nc.tensor.matmul(gs[j], qkT[:, tb, 0:S], qkT[:, tb, S:128],
                                     start=(tb < NACC), stop=(tb >= H - NACC))
                g_sb = p2one.tile([S, S], F32, tag="g_sb")
                nc.scalar.copy(g_sb, gs[0])
                for j in range(1, NACC):
                    nc.vector.tensor_tensor(g_sb, g_sb, gs[j], op=ALU.add)
                rqs = p2one.tile([S, 1], F32, tag="rqs")
                rks = p2one.tile([S, 1], F32, tag="rks")
                sq1 = p2one.tile([S, 1], F32, tag="sq1")
                sq2 = p2one.tile([S, 1], F32, tag="sq2")
                nc.vector.reciprocal_approx_fast(out=sq1, in_=rq2)
                nc.scalar.activation(rqs, sq1, AF.Sqrt)
                nc.vector.reciprocal_approx_fast(out=sq2, in_=rk2)
                nc.scalar.activation(rks, sq2, AF.Sqrt)
                temp_t = p2one.tile([S, 1], F32, tag="temp_t")
                nc.sync.dma_start(out=temp_t, in_=temp_in[:, :])
                nc.vector.tensor_tensor(rqs, rqs, temp_t, op=ALU.mult)
                nc.vector.tensor_scalar_mul(g_sb, g_sb, rqs)
                rk_row = p2one.tile([1, S], F32R, tag="rk_row")
                nc.sync.dma_start(out=rk_row, in_=rks[:, :].bitcast(F32R))
                ones1 = p2one.tile([1, S], F32R, tag="ones1")
                nc.sync.dma_start(out=ones1, in_=ones_in[:, :])
                rkb_ps = p2ps.tile([S, S], F32, tag="rkb_ps", bufs=1)
                nc.tensor.matmul(rkb_ps, ones1, rk_row, start=True, stop=True)
                s_sb = p2one.tile([S, 8, 8], F32, tag="s_sb")
                nc.vector.tensor_tensor(s_sb.rearrange("p a b -> p (a b)"), g_sb, rkb_ps, op=ALU.mult)
                mx = p2one.tile([S, 8], F32, tag="mx")
                nc.vector.reduce_max(mx, s_sb, axis=mybir.AxisListType.X)
                mxb = bass.AP(tensor=mx.tensor, offset=mx.offset,
                              ap=[list(mx.ap[0]), list(mx.ap[1]), [0, 8]])
                e_sb = p2one.tile([S, 8, 8], F32, tag="e_sb")
                nc.vector.tensor_tensor(e_sb, s_sb, mxb, op=ALU.subtract)
                ex_sb = p2one.tile([S, 8, 8], F32, tag="ex_sb")
                nc.scalar.activation(ex_sb, e_sb, AF.Exp)
                sm = p2one.tile([S, 8], F32, tag="sm")
                nc.vector.reduce_sum(sm, ex_sb, axis=mybir.AxisListType.X)
                rs = p2one.tile([S, 8], F32, tag="rs")
                nc.vector.reciprocal_approx_fast(out=rs, in_=sm)
                rsb = bass.AP(tensor=rs.tensor, offset=rs.offset,
                              ap=[list(rs.ap[0]), list(rs.ap[1]), [0, 8]])
                attn = p2one.tile([S, S], F32R, tag="attn")
                nc.vector.tensor_tensor(attn.rearrange("p (a b) -> p a b", a=8), ex_sb, rsb, op=ALU.mult)
                maskbd = p2one.tile([S, S], F32R, tag="maskbd")
                nc.sync.dma_start(out=maskbd, in_=mask_in[:, :])
                attn_m = p2one.tile([S, S], F32R, tag="attn_m")
                nc.vector.tensor_tensor(attn_m, attn, maskbd, op=ALU.mult)
                identr = p2one.tile([S, S], F32R, tag="identr")
                nc.sync.dma_start(out=identr, in_=identr_in[:, :])
                attn_tp = p2ps.tile([S, S], F32R, tag="attn_tp", bufs=1)
                nc.tensor.transpose(attn_tp, attn_m, identr)
                attn_t = p2one.tile([S, S], BF16, tag="attn_t")
                nc.scalar.copy(attn_t, attn_tp)
                for blk in range(HW // NPX):
                    ps_o = p2ps.tile([S, NPX], F32, tag="ps_o")
                    nc.tensor.matmul(ps_o, attn_t, vo_store[0:S, blk * NPX:(blk + 1) * NPX],
                                     start=True, stop=True)
                    nc.vector.tensor_copy(vo_store[S:128, blk * NPX:(blk + 1) * NPX], ps_o)

            # ================= PHASE 3: expand(+po) + LN + FFN =================
            with tc.tile_pool(name="p3", bufs=2) as p3, \
                 tc.tile_pool(name="p3ps", bufs=1, space="PSUM") as p3ps:

                def stage_a_pre(bslot, re, nrows, slo):
                    npx_e = nrows * W
                    s_m1, s_p1 = slo - 1, slo + 1
                    ps_e0 = p3ps.tile([128, NPX], F32, tag="ps_e0")
                    ps_e1 = p3ps.tile([128, NPX], F32, tag="ps_e1")
                    for dxi in range(3):
                        rhs_p = owin2[:, s_m1:s_m1 + nrows, dxi:dxi + W]
                        rhs_s = owin2[0:S, s_p1:s_p1 + nrows, dxi:dxi + W]
                        for mt, ps_e in ((0, ps_e0), (1, ps_e1)):
                            nc.tensor.matmul(ps_e[:, 0:npx_e],
                                             wexpa[:, dxi, mt * 128:(mt + 1) * 128], rhs_p,
                                             start=(dxi == 0), stop=False)
                            nc.tensor.matmul(ps_e[:, 0:npx_e],
                                             wexpb[:, dxi, mt * 128:(mt + 1) * 128], rhs_s,
                                             start=False, stop=(dxi == 2 and mt == 1))
                    nc.scalar.copy(t_strip[:, 0, bslot, 0:npx_e], ps_e0[:, 0:npx_e])
                    nc.scalar.copy(t_strip[:, 1, bslot, 0:npx_e], ps_e1[:, 0:npx_e])
                    sq0 = p3.tile([128, NPX], F32R, tag="sq0")
                    sq1t = p3.tile([128, NPX], F32R, tag="sq1t")
                    nc.scalar.activation(sq0[:, 0:npx_e], ps_e0[:, 0:npx_e], AF.Square)
                    nc.scalar.activation(sq1t[:, 0:npx_e], ps_e1[:, 0:npx_e], AF.Square)
                    ps_stm = p3ps.tile([1, NPX], F32, tag="small")
                    nc.tensor.matmul(ps_stm[0:1, 0:npx_e], stat256[:, 0:1],
                                     t_strip[:, 0, bslot, 0:npx_e], start=True, stop=False)
                    nc.tensor.matmul(ps_stm[0:1, 0:npx_e], stat256[:, 0:1],
                                     t_strip[:, 1, bslot, 0:npx_e], start=False, stop=True)
                    mu_t = p3.tile([1, NPX], BF16, tag="mu_t", bufs=5)
                    nc.vector.tensor_copy(mu_t[:, 0:npx_e], ps_stm[0:1, 0:npx_e])
                    ps_sts = p3ps.tile([1, NPX], F32, tag="small")
                    nc.tensor.matmul(ps_sts[0:1, 0:npx_e], stat256[:, 1:2],
                                     sq0[:, 0:npx_e], start=True, stop=False)
                    nc.tensor.matmul(ps_sts[0:1, 0:npx_e], stat256[:, 1:2],
                                     sq1t[:, 0:npx_e], start=False, stop=True)
                    musq = p3.tile([1, NPX], F32, tag="musq", bufs=1)
                    nc.gpsimd.tensor_tensor(musq[:, 0:npx_e], mu_t[:, 0:npx_e],
                                            mu_t[:, 0:npx_e], op=ALU.mult)
                    varr3 = p3.tile([1, NPX], F32, tag="varr3", bufs=2)
                    nc.vector.scalar_tensor_tensor(varr3[:, 0:npx_e],
                                                   ps_sts[0:1, 0:npx_e], EPS,
                                                   musq[:, 0:npx_e],
                                                   op0=ALU.add, op1=ALU.subtract)
                    rcp3 = p3.tile([1, NPX], F32, tag="rcp3", bufs=2)
                    nc.vector.reciprocal_approx_fast(out=rcp3[:, 0:npx_e],
                                                     in_=varr3[:, 0:npx_e])
                    r_t = p3.tile([1, NPX], BF16, tag="r_t", bufs=5)
                    nc.scalar.activation(r_t[:, 0:npx_e], rcp3[:, 0:npx_e], AF.Sqrt)
                    return mu_t, r_t

                def stage_a_vn(bslot, re, nrows, slo, mu_t, r_t):
                    npx_e = nrows * W
                    ps_bmu = p3ps.tile([128, NPX], F32, tag="bc", bufs=2)
                    nc.tensor.matmul(ps_bmu[:, 0:npx_e], ones128,
                                     mu_t[:, 0:npx_e], start=True, stop=True)
                    ps_br = p3ps.tile([128, NPX], F32, tag="bc", bufs=2)
                    nc.tensor.matmul(ps_br[:, 0:npx_e], ones128,
                                     r_t[:, 0:npx_e], start=True, stop=True)
                    vns = []
                    for h in range(2):
                        dh = p3.tile([128, NPX], F32, tag="dh")
                        nc.vector.tensor_tensor(dh[:, 0:npx_e], t_strip[:, h, bslot, 0:npx_e],
                                                ps_bmu[:, 0:npx_e], op=ALU.subtract)
                        vn = p3.tile([128, NPX], BF16, tag=f"vn{h}", bufs=5)
                        nc.vector.tensor_tensor(vn[:, 0:npx_e], dh[:, 0:npx_e],
                                                ps_br[:, 0:npx_e], op=ALU.mult)
                        vns.append(vn)
                    return vns

                def stage_a_f1(bslot, re, nrows, slo, vns):
                    npx_e = nrows * W
                    for mt in range(2):
                        ps_f = p3ps.tile([128, NPX], F32, tag="work", bufs=2)
                        nc.tensor.matmul(ps_f[:, 0:npx_e], wf1v[:, 0, mt],
                                         vns[0][:, 0:npx_e], start=True, stop=False)
                        nc.tensor.matmul(ps_f[:, 0:npx_e], wf1v[:, 1, mt],
                                         vns[1][:, 0:npx_e], start=False, stop=True)
                        fxs = p3.tile([128, NPX], BF16, tag="fxs")
                        nc.sync.dma_start(out=fxs[:, 0:npx_e],
                                          in_=fx_dram[mt, :, re * W:re * W + npx_e])
                        dstf = f1win[:, mt, slo + 1:slo + 1 + nrows, 1:1 + W]
                        nc.vector.tensor_tensor(
                            dstf, ps_f[:, 0:npx_e].rearrange("p (a b) -> p a b", a=nrows),
                            fxs[:, 0:npx_e].rearrange("p (a b) -> p a b", a=nrows),
                            op=ALU.add)

                def stage_b_dw(rg, nrg, slg):
                    npx_g = nrg * W
                    gsb = p3.tile([128, 2, NPX], BF16, tag="gsb", bufs=4)
                    for ct in range(2):
                        ps_g = p3ps.tile([128, NPX], F32, tag="work", bufs=2)
                        for t_i, (dy, dx) in enumerate(TAPS):
                            rhs = f1win[:, ct, slg + dy:slg + dy + nrg, 1 + dx:1 + dx + W]
                            nc.tensor.matmul(ps_g[:, 0:npx_g], wdw[:, ct, t_i], rhs,
                                             start=(t_i == 0), stop=(t_i == 8))
                        nc.scalar.activation(gsb[:, ct, 0:npx_g], ps_g[:, 0:npx_g], AF.Gelu,
                                             bias=bias_g[:, ct])
                    return gsb

                def stage_b_f2(rg, nrg, slg, gsb):
                    npx_g = nrg * W
                    for mt in range(2):
                        ps_out = p3ps.tile([128, NPX], F32, tag="work", bufs=2)
                        nc.tensor.matmul(ps_out[:, 0:npx_g], wf2[:, 0, mt], gsb[:, 0, 0:npx_g],
                                         start=True, stop=False)
                        nc.tensor.matmul(ps_out[:, 0:npx_g], wf2[:, 1, mt], gsb[:, 1, 0:npx_g],
                                         start=False, stop=True)
                        osb = p3.tile([128, NPX], F32, tag="osb")
                        nc.scalar.copy(osb[:, 0:npx_g], ps_out[:, 0:npx_g])
                        nc.sync.dma_start(out=out_dram[mt, :, rg * W:rg * W + npx_g],
                                          in_=osb[:, 0:npx_g])

                for s_i in range(NSTRIP):
                    r0 = 16 * s_i
                    last = (s_i == NSTRIP - 1)
                    if s_i > 0:
                        nc.gpsimd.tensor_copy(owin2[:, 0:2], owin2[:, 16:18])
                        nc.gpsimd.tensor_copy(f1win[:, :, 0:3], f1win[:, :, 16:19])
                    blocks = []
                    for b_i in range(NBLK):
                        rb = r0 + BLK_ROWS * b_i
                        sl_w = 4 * b_i + 2
                        src = vo_store[S:128, rb * W:(rb + 4) * W]
                        nc.vector.tensor_copy(owin2[0:S, sl_w:sl_w + 4, 1:1 + W],
                                              src.rearrange("p (a b) -> p a b", a=4))
                        nc.gpsimd.tensor_copy(owin2[S:128, sl_w - 1:sl_w + 3, 1:1 + W],
                                              src.rearrange("p (a b) -> p a b", a=4))
                        if s_i == 0 and b_i == 0:
                            blk = (b_i, 0, 3, 2)
                        else:
                            blk = (b_i, rb - 1, 4, 4 * b_i + 1)
                        mu_t, r_t = stage_a_pre(*blk)
                        blocks.append((blk, mu_t, r_t))
                    if last:
                        blk = (4, 127, 1, 17)
                        mu_t, r_t = stage_a_pre(*blk)
                        blocks.append((blk, mu_t, r_t))
                    for blk, mu_t, r_t in blocks:
                        vns = stage_a_vn(*blk, mu_t, r_t)
                        stage_a_f1(*blk, vns)
                    for b_i in range(NBLK):
                        rb = r0 + BLK_ROWS * b_i
                        if s_i == 0 and b_i == 0:
                            a = (0, 2, 3)
                        else:
                            a = (rb - 2, 4, 4 * b_i + 1)
                        gsb = stage_b_dw(*a)
                        stage_b_f2(*a, gsb)
                a = (126, 2, 17)
                gsb = stage_b_dw(*a)
                stage_b_f2(*a, gsb)
            p3w_stack.close()
    return nc


def _prep_host(inputs):
    f32 = np.float32
    w_cq = np.asarray(inputs["w_cq"], f32)
    w_ckv = np.asarray(inputs["w_ckv"], f32)
    ln_q_w = np.asarray(inputs["ln_q_w"], f32); ln_q_b = np.asarray(inputs["ln_q_b"], f32)
    ln_kv_w = np.asarray(inputs["ln_kv_w"], f32); ln_kv_b = np.asarray(inputs["ln_kv_b"], f32)
    w_kv = np.asarray(inputs["w_kv"], f32)
    w_kvdw = np.asarray(inputs["w_kvdw"], f32)
    w_q = np.asarray(inputs["w_q"], f32)
    temperature = np.asarray(inputs["temperature"], f32)
    w_po = np.asarray(inputs["w_po"], f32)
    w_expand = np.asarray(inputs["w_expand"], f32)
    ln_out_w = np.asarray(inputs["ln_out_w"], f32); ln_out_b = np.asarray(inputs["ln_out_b"], f32)
    w_ffn1 = np.asarray(inputs["w_ffn1"], f32)
    w_ffn_dw = np.asarray(inputs["w_ffn_dw"], f32)
    w_ffn2 = np.asarray(inputs["w_ffn2"], f32)

    d = {}

    def conv_lhsT(wc, scale=1.0):
        a = np.zeros((128, 2, 9, S + 1), f32)
        for kt in range(2):
            blk = wc[:, kt * 128:(kt + 1) * 128]           # [S, 128, 3, 3]
            a[:, kt, :, :S] = blk.transpose(1, 2, 3, 0).reshape(128, 9, S)
            a[:, kt, :, S] = blk.sum(axis=0).reshape(128, 9)
        return a * scale
    wcq8 = np.zeros((128, 9, 2, 128), f32)
    wcq8[:, :, :, :S + 1] = conv_lhsT(w_cq, SC8).transpose(0, 2, 1, 3)
    d["wcq8"] = wcq8.astype(F8NP)
    d["wckv"] = conv_lhsT(w_ckv)

    # q conv (fp8, scaled): K=128 DoubleRow layout over (low/high rows x 2 k-tiles)
    # low partitions see row(slot)+i, high partitions see row(slot)+1+i;
    # k-tile1 reads one slot later. dy=-1 -> (k0,low), dy=0 -> (k0,high),
    # dy=+1 -> (k1,high); (k1,low) gets zero weights.
    wq = (w_q * ln_q_w[None, :, None, None]) * SC8        # [S out, S in, 3, 3]
    wq2 = np.zeros((128, 3, 2, S), f32)
    for dxi in range(3):
        wq2[0:S, dxi, 0, :] = wq[:, :, 0, dxi].T
        wq2[S:128, dxi, 0, :] = wq[:, :, 1, dxi].T
        wq2[S:128, dxi, 1, :] = wq[:, :, 2, dxi].T
    d["wq2"] = wq2.astype(F8NP)
    d["bias_q"] = ((w_q * ln_q_b[None, :, None, None]).sum(axis=(1, 2, 3)) * SC8).reshape(S, 1)

    # kv conv (f32r, dy-stacked)
    w_kv_g = w_kv[:, :, 0, 0] * ln_kv_w[None, :]
    w_kv_eff = w_kvdw[:, 0][:, None] * w_kv_g[:, :, None, None]   # [2S, S, 3, 3]
    d["bias_kv"] = (w_kvdw[:, 0].sum(axis=(1, 2)) * (w_kv[:, :, 0, 0] @ ln_kv_b)).reshape(2 * S, 1)
    wkva = np.zeros((128, 3, 128), f32)
    wkvb = np.zeros((S, 3, 128), f32)
    for dxi in range(3):
        wkva[0:S, dxi, :] = w_kv_eff[:, :, 0, dxi].T
        wkva[S:128, dxi, :] = w_kv_eff[:, :, 1, dxi].T
        wkvb[:, dxi, :] = w_kv_eff[:, :, 2, dxi].T
    d["wkva"] = wkva
    d["wkvb"] = wkvb

    # expand conv folded with project_out (bf16, dy-stacked)
    wpo_m = w_po[:, :, 0, 0]                              # [S out(i), S in(j)]
    w_eff = np.einsum("cit,ij->cjt", w_expand.reshape(C, S, 9), wpo_m).reshape(C, S, 3, 3)
    wexpa = np.zeros((128, 3, C), f32)
    wexpb = np.zeros((S, 3, C), f32)
    for dxi in range(3):
        wexpa[0:S, dxi, :] = w_eff[:, :, 0, dxi].T
        wexpa[S:128, dxi, :] = w_eff[:, :, 1, dxi].T
        wexpb[:, dxi, :] = w_eff[:, :, 2, dxi].T
    d["wexpa"] = wexpa.astype(ml_dtypes.bfloat16)
    d["wexpb"] = wexpb.astype(ml_dtypes.bfloat16)

    w1 = w_ffn1[:, :, 0, 0]
    w1x = w1[:, :C]
    w1v = w1[:, C:] * ln_out_w[None, :]

    def one_by_one_lhsT(wm):
        a = np.zeros((128, 2, 2, 128), f32)
        for kt in range(2):
            for mt in range(2):
                a[:, kt, mt, :] = wm[mt * 128:(mt + 1) * 128, kt * 128:(kt + 1) * 128].T
        return a
    d["wf1x"] = one_by_one_lhsT(w1x)
    d["wf1v"] = one_by_one_lhsT(w1v).astype(ml_dtypes.bfloat16)
    bias_f1 = w1[:, C:] @ ln_out_b
    dw_t = w_ffn_dw[:, 0].reshape(C, 9)
    d["bias_g"] = np.ascontiguousarray(
        (bias_f1 * dw_t.sum(1)).reshape(2, 128, 1).transpose(1, 0, 2))
    wdw = np.zeros((128, 2, 9, 128), f32)
    for ct in range(2):
        for t in range(9):
            np.fill_diagonal(wdw[:, ct, t, :], dw_t[ct * 128:(ct + 1) * 128, t])
    d["wdw"] = wdw.astype(ml_dtypes.bfloat16)
    d["wf2"] = one_by_one_lhsT(w_ffn2[:, :, 0, 0]).astype(ml_dtypes.bfloat16)
    stat_cq = np.zeros((S + 1, 1), f32)
    stat_cq[:S, 0] = 1.0 / S
    stat_cq[S, 0] = -1.0 / (S * S)
    d["stat_cq"] = stat_cq
    # brs row0 = rsqrt (Act-written), row32 = raw channel sum (DVE-written)
    bc2 = np.zeros((33, 128), f32)
    bc2[0, S:128] = 1.0
    bc2[32, 0:S] = 1.0 / S
    d["bc2"] = bc2
    d["ones128"] = np.ones((1, 128), f32).astype(ml_dtypes.bfloat16)
    stat256 = np.zeros((128, 2), f32)
    stat256[:, 0] = 1.0 / C
    stat256[:, 1] = 1.0 / C
    d["stat256"] = stat256
    d["tempv"] = np.repeat(temperature.reshape(HEADS), S // HEADS).reshape(S, 1).astype(f32)
    mask = np.zeros((S, S), f32)
    for h in range(HEADS):
        mask[h * 8:(h + 1) * 8, h * 8:(h + 1) * 8] = 1.0
    d["maskbd"] = mask
    d["identr"] = np.eye(S, dtype=f32)
    d["identm"] = np.eye(128, dtype=f32).astype(ml_dtypes.bfloat16)
    d["zeros8"] = np.zeros((128, 2600), f32).astype(F8NP)
    d["zeros"] = np.zeros((128, 2600), f32)
    d["zerosb"] = np.zeros((128, 2600), f32).astype(ml_dtypes.bfloat16)
    d["onesr"] = np.ones((1, S), f32)
    return d


def _pad_input(x):
    """[C,H,W] f32 -> [128, 2, H+4, WP] zero-padded, partition-major"""
    out = np.zeros((128, 2, H + 4, WP), np.float32)
    out[:, :, 2:H + 2, 1:W + 1] = x.reshape(2, 128, H, W).transpose(1, 0, 2, 3)
    return out


def make_in_maps(inputs):
    d = _prep_host(inputs)
    x = np.asarray(inputs["x"], np.float32)
    y = np.asarray(inputs["y"], np.float32)
    in_maps = []
    for i in range(B):
        m = dict(d)
        xp = _pad_input(x[i])
        m["x"] = xp
        m["x8"] = xp.astype(F8NP)
        m["y"] = _pad_input(y[i])
        in_maps.append(m)
    return in_maps


def kernel(**inputs):
    key = "nc"
    if key not in _CACHED:
        nc = build_nc()
        nc.finalize()
        _CACHED[key] = nc
    nc = _CACHED[key]
    in_maps = make_in_maps(inputs)
    res = run_bass_kernel_spmd(nc, in_maps, list(range(B)))
    out = np.stack([res.results[i]["out"].reshape(C, H, W) for i in range(B)])
    return out.astype(np.float32)
